# revision 1
# baseline (speedup 1.0000x reference)
"""GQA causal attention (ternary weights) on 8 TRN2 NeuronCores.

Strategy (tensor-parallel over heads, per sharding hint):
  - core c owns Q heads [4c, 4c+4) and KV head c.
  - host: ternarize weights, transpose + fp16-hi/lo-split x (exact 2^10
    compensation so fp16 subnormal FTZ cannot hurt), slice weights per core.
  - device per core:
      phase 1: q/k/v projections as 2-pass fp16 matmuls (ternary weights are
               exact in fp16; lo pass uses 2^10-scaled x residual against
               2^-10-scaled weights).
      phase 2: per (batch, head): cheap 1-pass fp16 S=QK^T in [q,k] layout for
               a row-max estimate (software-pipelined one head ahead); then
               exact-enough S^T in [k,q] layout via two matmuls (hi pass with
               folded -max bias row, compensated residual pass), exp on
               ScalarE, and PV + row-sums via a single fp32r matmul with a
               ones column appended to V.
      phase 3: o_proj partial (this core's 256 input dims) via fp32r,
               emitted per batch so its DMA overlaps the other batch's
               attention.
  - host: sum the 8 partial outputs (the "all-reduce" of the row-split o_proj).
"""

import sys

sys.path.insert(0, "/opt/trn_rl_repo")

import numpy as np

B = 2
S = 2048
D = 2048
NCORES = 8
HEADS_PER_CORE = 4
HD = 64
QROWS = HEADS_PER_CORE * HD  # 256
TT = 512  # token tile
LO_SCALE = 1024.0  # 2**10 subnormal-avoidance scale for fp16 lo pieces
MASK_NEG = -30000.0

_CACHE = {}


def _build_program(b=B, s=S, d=D):
    import concourse.bacc as bacc
    import concourse.tile as tile
    import concourse.mybir as mybir
    from concourse import masks
    from contextlib import ExitStack

    f32 = mybir.dt.float32
    f32r = mybir.dt.float32r
    f16 = mybir.dt.float16
    Alu = mybir.AluOpType
    Act = mybir.ActivationFunctionType

    tokens = b * s
    n_tt = tokens // TT          # token tiles
    tt_per_b = s // TT
    n_dc = d // 128              # contraction chunks for projections
    n_qt = s // TT               # 512-wide q tiles per batch
    n_qc = s // 128              # 128-wide q chunks per batch (max pass)
    n_mt = d // 128              # output row tiles for o_proj
    n_oc = QROWS // 128          # o_proj contraction chunks (2)
    sub = TT // 128              # 128-sub-blocks per 512 tile (4)

    nc = bacc.Bacc("TRN2", target_bir_lowering=False, debug=False,
                   num_devices=NCORES)

    xh_d = nc.dram_tensor("xh", [d, tokens], f16, kind="ExternalInput").ap()
    xl_d = nc.dram_tensor("xl", [d, tokens], f16, kind="ExternalInput").ap()
    wqh_d = nc.dram_tensor("wq_hi", [d, QROWS], f16, kind="ExternalInput").ap()
    wql_d = nc.dram_tensor("wq_lo", [d, QROWS], f16, kind="ExternalInput").ap()
    wkh_d = nc.dram_tensor("wkv_hi", [d, 128], f16, kind="ExternalInput").ap()
    wkl_d = nc.dram_tensor("wkv_lo", [d, 128], f16, kind="ExternalInput").ap()
    wo_d = nc.dram_tensor("wo", [QROWS, d], f32r, kind="ExternalInput").ap()
    out_d = nc.dram_tensor("out", [d, tokens], f32, kind="ExternalOutput").ap()

    with tile.TileContext(nc) as tc, ExitStack() as top:
        constp = top.enter_context(tc.tile_pool(name="const", bufs=1))
        wpool = top.enter_context(tc.tile_pool(name="wts", bufs=1))
        pp = top.enter_context(tc.tile_pool(name="persist", bufs=1))

        # --- constants -------------------------------------------------
        maskM = constp.tile([128, 128], f32, tag="maskM")   # [k,q] diag: keep k<=q
        nc.gpsimd.memset(maskM[:], 0.0)
        nc.gpsimd.affine_select(
            out=maskM[:], in_=maskM[:], compare_op=Alu.is_ge, fill=MASK_NEG,
            base=0, pattern=[[1, 128]], channel_multiplier=-1)
        maskM2 = constp.tile([128, 128], f32, tag="maskM2")  # [q,k] diag: keep k<=q
        nc.gpsimd.memset(maskM2[:], 0.0)
        nc.gpsimd.affine_select(
            out=maskM2[:], in_=maskM2[:], compare_op=Alu.is_ge, fill=MASK_NEG,
            base=0, pattern=[[-1, 128]], channel_multiplier=1)
        ident = constp.tile([128, 128], f32, tag="ident")
        masks.make_identity(nc, ident[:])
        onesc = constp.tile([65, HD], f32r, tag="onesc")
        nc.scalar.activation(onesc[:], maskM[0:65, 0:HD], Act.Identity,
                             bias=1.0, scale=0.0)

        # --- weights ---------------------------------------------------
        wq_sb = {}
        for name, dram in (("hi", wqh_d), ("lo", wql_d)):
            t = wpool.tile([128, n_dc * QROWS], f16, tag=f"wq{name}",
                           name=f"wq{name}")
            nc.sync.dma_start(
                out=t[:].rearrange("p (c n) -> p c n", n=QROWS),
                in_=dram.rearrange("(c p) n -> p c n", p=128))
            wq_sb[name] = t
        wkv_sb = {}
        for name, dram in (("hi", wkh_d), ("lo", wkl_d)):
            t = wpool.tile([128, n_dc * 128], f16, tag=f"wkv{name}",
                           name=f"wkv{name}")
            nc.sync.dma_start(
                out=t[:].rearrange("p (c n) -> p c n", n=128),
                in_=dram.rearrange("(c p) n -> p c n", p=128))
            wkv_sb[name] = t

        # --- persistent activations -----------------------------------
        # qA[h]: rows 0:64 = fp16(q/8) "qh", row 64 = m~ bias (max pass)
        # qB[h]: rows 0:64 = qh * 2^-10, rows 64:128 = fp16(2^10 * (q/8 - qh))
        # khb:   rows 0:64 = fp16(k) "kh", row 64 = -1
        # klkh:  rows 0:64 = fp16(2^10 * (k - kh)), rows 64:128 = kh * 2^-10
        # vhat:  [128, chunk*65]: cols 0:64 of chunk = v (natural layout),
        #        col 64 = 1.0
        qA = [pp.tile([65, tokens], f16, tag=f"qA{h}", name=f"qA{h}")
              for h in range(HEADS_PER_CORE)]
        qB = [pp.tile([128, tokens], f16, tag=f"qB{h}", name=f"qB{h}")
              for h in range(HEADS_PER_CORE)]
        khb = pp.tile([65, tokens], f16, tag="khb")
        klkh = pp.tile([128, tokens], f16, tag="klkh")
        n_ch = tokens // 128
        vhat = pp.tile([128, n_ch * 65], f32r, tag="vhat")
        nc.scalar.activation(
            vhat[:], maskM[:, 0:1].to_broadcast([128, n_ch * 65]),
            Act.Identity, bias=1.0, scale=0.0)
        nc.gpsimd.memset(khb[64:65, :], -1.0)

        with ExitStack() as ph:
            mp = ph.enter_context(tc.tile_pool(name="mp", bufs=2))
            ps1 = ph.enter_context(
                tc.tile_pool(name="ps1", bufs=3, space="PSUM"))
            psst = ph.enter_context(
                tc.tile_pool(name="psst", bufs=2, space="PSUM"))
            psav = ph.enter_context(
                tc.tile_pool(name="psav", bufs=2, space="PSUM"))
            psbc = ph.enter_context(
                tc.tile_pool(name="psbc", bufs=1, space="PSUM"))

            # ---------- S~ max-estimate pass, as schedulable blocks ------
            mstate = {}
            mbp = ph.enter_context(tc.tile_pool(name="mbp", bufs=8))

            def s_block(bb, h, qc):
                boff = bb * s
                if qc == 0:
                    mstate[(bb, h)] = mbp.tile([128, n_qc], f32, tag="mbuf",
                                               name="mbuf")
                mbuf = mstate[(bb, h)]
                qsl = slice(boff + qc * 128, boff + qc * 128 + 128)
                ntk = qc // sub + 1
                mtmp = mp.tile([128, 8], f32, tag="mtmp")
                for kt in range(ntk):
                    w = min(TT, (qc + 1) * 128 - kt * TT)
                    st = psst.tile([128, TT], f32, tag="st")
                    nc.tensor.matmul(
                        st[:, 0:w],
                        lhsT=qA[h][0:64, qsl],
                        rhs=khb[0:64, boff + kt * TT:boff + kt * TT + w],
                        start=True, stop=True)
                    if kt == ntk - 1:  # diagonal block is last 128 cols
                        nc.vector.tensor_tensor(
                            st[:, w - 128:w], st[:, w - 128:w],
                            maskM2[:], op=Alu.add)
                    nc.vector.tensor_reduce(
                        mtmp[:, kt:kt + 1], st[:, 0:w],
                        axis=mybir.AxisListType.X, op=Alu.max)
                nc.vector.tensor_reduce(
                    mbuf[:, qc:qc + 1], mtmp[:, 0:ntk],
                    axis=mybir.AxisListType.X, op=Alu.max)

            def s_final(bb, h):
                boff = bb * s
                mbuf = mstate.pop((bb, h))
                mps = psst.tile([128, TT], f32, tag="st")
                nc.tensor.transpose(mps[0:n_qc, 0:128], mbuf[:, 0:n_qc],
                                    ident[:, 0:128])
                mrow = mp.tile([n_qc, 128], f32, tag="mrow")
                nc.vector.tensor_copy(mrow[:], mps[0:n_qc, 0:128])
                nc.gpsimd.dma_start(
                    out=qA[h][64:65, boff:boff + s].rearrange(
                        "o (c t) -> o c t", t=128),
                    in_=mrow[:])


            # ================= phase 1: projections ====================
            ph1 = ExitStack()
            xp = ph1.enter_context(tc.tile_pool(name="xp", bufs=3))
            sp1 = ph1.enter_context(tc.tile_pool(name="sp1", bufs=3))
            for tt in range(n_tt):
                tcols = slice(tt * TT, (tt + 1) * TT)
                x_sb = {}
                for name, dram in (("hi", xh_d), ("lo", xl_d)):
                    t = xp.tile([128, n_dc * TT], f16, tag="x", name="xtile")
                    nc.sync.dma_start(
                        out=t[:].rearrange("p (c t) -> p c t", t=TT),
                        in_=dram.rearrange("(c p) t -> p c t",
                                           p=128)[:, :, tcols])
                    x_sb[name] = t

                def proj(w_sb, mcol, mwid, ps):
                    first = True
                    for name in ("hi", "lo"):
                        for c in range(n_dc):
                            nc.tensor.matmul(
                                ps[:],
                                lhsT=w_sb[name][:, c * mwid + mcol:
                                                c * mwid + mcol + 128],
                                rhs=x_sb[name][:, c * TT:(c + 1) * TT],
                                start=first,
                                stop=(name == "lo" and c == n_dc - 1))
                            first = False

                for m in range(QROWS // 128):
                    ps = ps1.tile([128, TT], f32, tag="ps")
                    proj(wq_sb, m * 128, QROWS, ps)
                    tmp = sp1.tile([128, TT], f32, tag="qtmp")
                    nc.scalar.mul(tmp[:], ps[:], 0.125)
                    qh16 = sp1.tile([128, TT], f16, tag="qh16")
                    nc.scalar.copy(qh16[:], tmp[:])
                    res = sp1.tile([128, TT], f32, tag="qres")
                    nc.vector.tensor_tensor(
                        res[:], tmp[:], qh16[:], op=Alu.subtract)
                    for i in range(2):
                        h = 2 * m + i
                        rows = slice(i * 64, i * 64 + 64)
                        nc.scalar.copy(qA[h][0:64, tcols], qh16[rows, :])
                        nc.scalar.mul(qB[h][0:64, tcols], qh16[rows, :],
                                      1.0 / LO_SCALE)
                        nc.scalar.mul(qB[h][64:128, tcols], res[rows, :],
                                      LO_SCALE)

                ps = ps1.tile([128, TT], f32, tag="ps")
                proj(wkv_sb, 0, 128, ps)
                nc.scalar.copy(khb[0:64, tcols], ps[0:64, :])
                nc.scalar.mul(klkh[64:128, tcols], ps[0:64, :], 1.0 / LO_SCALE)
                res = sp1.tile([128, TT], f32, tag="qres")
                nc.vector.tensor_tensor(
                    res[0:64, :], ps[0:64, :], khb[0:64, tcols],
                    op=Alu.subtract)
                nc.scalar.mul(klkh[0:64, tcols], res[0:64, :], LO_SCALE)
                vtmp = sp1.tile([64, TT], f32, tag="vtmp")
                nc.scalar.copy(vtmp[:], ps[64:128, :])
                for j in range(sub):
                    ptr = psst.tile([128, TT], f32, tag="st")
                    nc.tensor.transpose(ptr[0:128, 0:64],
                                        vtmp[:, j * 128:(j + 1) * 128],
                                        ident[0:64, 0:64])
                    ch = tt * sub + j
                    nc.scalar.copy(vhat[:, ch * 65:ch * 65 + 64],
                                   ptr[0:128, 0:64])
                # batch-0 S~ blocks for the q-chunks this tile enabled
                bb, ltt = tt // tt_per_b, tt % tt_per_b
                if bb == 0:
                    for h in range(HEADS_PER_CORE):
                        for qc in range(ltt * sub, (ltt + 1) * sub):
                            s_block(bb, h, qc)
                    if ltt == tt_per_b - 1:
                        for h in range(HEADS_PER_CORE):
                            s_final(bb, h)

            ph1.close()
            # ============ phase 2 + per-batch o_proj ====================
            aop = ph.enter_context(tc.tile_pool(name="aop", bufs=1))
            ptp = ph.enter_context(tc.tile_pool(name="ptp", bufs=4))
            outp = ph.enter_context(tc.tile_pool(name="outp", bufs=2))
            ao = [aop.tile([128, tokens], f32r, tag=f"ao{i}", name=f"ao{i}")
                  for i in range(2)]
            wo_sb = aop.tile([128, n_oc * d], f32r, tag="wo")
            nc.sync.dma_start(
                out=wo_sb[:].rearrange("p (c n) -> p c n", n=d),
                in_=wo_d.rearrange("(c p) n -> p c n", p=128))

            def av(pav, pt, lo, w, bb, kc, nchunks):
                ch = bb * (s // 128) + kc
                nc.tensor.matmul(
                    pav[:, lo:lo + w], lhsT=vhat[:, ch * 65:ch * 65 + 65],
                    rhs=pt[:, lo:lo + w],
                    start=(kc == 0), stop=(kc == nchunks - 1),
                    skip_group_check=True)


            def oproj_m(bb, m):
                boff = bb * s
                osb = outp.tile([128, s], f32, tag="ot")
                for tt2 in range(tt_per_b):
                    po = ps1.tile([128, TT], f32, tag="ps")
                    for ci in range(n_oc):
                        nc.tensor.matmul(
                            po[:],
                            lhsT=wo_sb[:, ci * d + m * 128:
                                       ci * d + m * 128 + 128],
                            rhs=ao[ci][:, boff + tt2 * TT:
                                       boff + (tt2 + 1) * TT],
                            start=(ci == 0), stop=(ci == n_oc - 1))
                    nc.vector.tensor_copy(osb[:, tt2 * TT:(tt2 + 1) * TT],
                                          po[:])
                nc.sync.dma_start(
                    out=out_d[m * 128:(m + 1) * 128, boff:boff + s],
                    in_=osb[:])

            b1q = []
            for h in range(HEADS_PER_CORE):
                for qc in range(n_qc):
                    b1q.append(lambda h=h, qc=qc: s_block(1, h, qc))
                b1q.append(lambda h=h: s_final(1, h))
            # weight S~ pacing by main-slot size (qt+1 chunks of work)
            wsum = HEADS_PER_CORE * n_qt * (n_qt + 1) // 2
            bcum, acc = [], 0.0
            for h in range(HEADS_PER_CORE):
                for qt in range(n_qt):
                    acc += (qt + 1) * len(b1q) / wsum
                    bcum.append(min(int(round(acc)), len(b1q)))
            bcum[-1] = len(b1q)

            def oproj_part(bb, m, qt0, nqt):
                boff = bb * s
                osb = outp.tile([128, nqt * TT], f32, tag="ot", name="osb")
                for i in range(nqt):
                    po = ps1.tile([128, TT], f32, tag="ps")
                    for ci in range(n_oc):
                        nc.tensor.matmul(
                            po[:],
                            lhsT=wo_sb[:, ci * d + m * 128:
                                       ci * d + m * 128 + 128],
                            rhs=ao[ci][:, boff + (qt0 + i) * TT:
                                       boff + (qt0 + i + 1) * TT],
                            start=(ci == 0), stop=(ci == n_oc - 1))
                    nc.vector.tensor_copy(osb[:, i * TT:(i + 1) * TT], po[:])
                nc.sync.dma_start(
                    out=out_d[m * 128:(m + 1) * 128,
                              boff + qt0 * TT:boff + (qt0 + nqt) * TT],
                    in_=osb[:])

            slot = 0
            for bb in range(b):
                boff = bb * s
                for h in range(HEADS_PER_CORE):
                    for qt in range(n_qt):
                        qlo = boff + qt * TT
                        pav = psav.tile([65, TT], f32, tag="pav")
                        nchunks = (qt + 1) * sub
                        prev = None
                        for kc in range(nchunks):
                            ksl = slice(boff + kc * 128, boff + kc * 128 + 128)
                            j = kc - qt * sub
                            lo = max(j, 0) * 128  # cols < lo fully masked
                            w = TT - lo
                            s2 = ps1.tile([128, TT], f32, tag="ps")
                            nc.tensor.matmul(
                                s2[:, lo:lo + w], lhsT=khb[:, ksl],
                                rhs=qA[h][:, qlo + lo:qlo + TT],
                                start=True, stop=False)
                            nc.tensor.matmul(
                                s2[:, lo:lo + w], lhsT=klkh[:, ksl],
                                rhs=qB[h][:, qlo + lo:qlo + TT],
                                start=False, stop=True)
                            if j >= 0:
                                nc.vector.tensor_tensor(
                                    s2[:, lo:lo + 128], s2[:, lo:lo + 128],
                                    maskM[:], op=Alu.add)
                            pt = ptp.tile([128, TT], f32r, tag="pt")
                            nc.scalar.activation(pt[:, lo:lo + w],
                                                 s2[:, lo:lo + w], Act.Exp)
                            if prev is not None:
                                av(pav, prev[0], prev[1], prev[2], bb,
                                   prev[3], nchunks)
                            prev = (pt, lo, w, kc)
                        av(pav, prev[0], prev[1], prev[2], bb, prev[3],
                           nchunks)

                        rec = mp.tile([65, TT], f32r, tag="rec")
                        with nc.allow_low_precision(
                                reason="1/l broadcast feeds fp32r matmul"):
                            nc.vector.reciprocal(rec[64:65, :],
                                                 pav[64:65, :])
                        bc = psbc.tile([64, TT], f32, tag="bc")
                        nc.tensor.matmul(
                            bc[:], lhsT=onesc[64:65, 0:HD],
                            rhs=rec[64:65, :], start=True, stop=True)
                        bcs = mp.tile([64, TT], f32, tag="bcs")
                        nc.scalar.copy(bcs[:], bc[:])
                        rows = slice((h % 2) * 64, (h % 2) * 64 + 64)
                        nc.vector.tensor_tensor(
                            ao[h // 2][rows, qlo:qlo + TT], pav[0:64, :],
                            bcs[:], op=Alu.mult)

                        if bb == 0:
                            # batch-1 S~ rides along, weighted by slot size
                            lo_i = bcum[slot - 1] if slot else 0
                            for fn in b1q[lo_i:bcum[slot]]:
                                fn()
                            slot += 1
                        elif True:
                            # b0 o_proj spread over b1 slots (any head), and
                            # b1 o_proj streamed behind the last head
                            si = (h * n_qt + qt)
                            per = (n_mt + HEADS_PER_CORE * n_qt - 1) \
                                // (HEADS_PER_CORE * n_qt)
                            for m in range(si * per,
                                           min((si + 1) * per, n_mt)):
                                oproj_part(0, m, 0, n_qt)
            for m in range(n_mt):
                oproj_part(1, m, 0, n_qt)

    nc.compile()
    return nc


def _ternarize(w):
    w = np.asarray(w, np.float32)
    scale = max(np.abs(w).mean(), 1e-6)
    return ((w > 0.05 * scale).astype(np.float32)
            - (w < -0.05 * scale).astype(np.float32))


def _split_f16(a):
    hi = a.astype(np.float16)
    lo = ((a - hi.astype(np.float32)) * LO_SCALE).astype(np.float16)
    return hi, lo


def kernel(x, wq, wk, wv, wo):
    from concourse.bass_utils import run_bass_kernel_spmd

    if "nc" not in _CACHE:
        _CACHE["nc"] = _build_program()
    nc = _CACHE["nc"]

    tq = _ternarize(wq)
    tk = _ternarize(wk)
    tv = _ternarize(wv)
    to = _ternarize(wo)

    xT = np.ascontiguousarray(np.asarray(x, np.float32).reshape(B * S, D).T)
    xh, xl = _split_f16(xT)

    in_maps = []
    for c in range(NCORES):
        qsl = slice(c * QROWS, (c + 1) * QROWS)
        ksl = slice(c * HD, (c + 1) * HD)
        wkv = np.concatenate([tk[ksl], tv[ksl]], axis=0)  # [128, D]
        in_maps.append({
            "xh": xh, "xl": xl,
            "wq_hi": np.ascontiguousarray(tq[qsl].T).astype(np.float16),
            "wq_lo": np.ascontiguousarray(tq[qsl].T / LO_SCALE).astype(np.float16),
            "wkv_hi": np.ascontiguousarray(wkv.T).astype(np.float16),
            "wkv_lo": np.ascontiguousarray(wkv.T / LO_SCALE).astype(np.float16),
            "wo": np.ascontiguousarray(to[:, qsl].T).astype(np.float32),
        })

    res = run_bass_kernel_spmd(nc, in_maps, list(range(NCORES)))
    total = res.results[0]["out"]
    for c in range(1, NCORES):
        total = total + res.results[c]["out"]
    return np.ascontiguousarray(total.T).reshape(B, S, D).astype(np.float32)



# revision 14
# speedup vs baseline: 1.1326x; 1.1326x over previous
"""GQA causal attention (ternary weights) on 8 TRN2 NeuronCores.

Strategy (tensor-parallel over heads, per sharding hint):
  - core c owns Q heads [4c, 4c+4) and KV head c.
  - host: ternarize weights; split x into fp16 hi + fp8(e4m3, x512) residual;
    fp8 weight copies are ternary/512 (exact: 2^-9 is an e4m3 subnormal) so
    the fp8 DoubleRow lo-pass lands at natural scale in the same PSUM group
    as the fp16 hi-pass.
  - device per core:
      phase 1: q/k/v projections: fp16 hi matmuls + fp8 DoubleRow lo matmuls
               (2 contraction chunks per instruction, half cost). Activations
               are stored as fp16 (qA rows 0:64 = q/8, khb rows 0:64 = k,
               row 64 = bias) plus fp8 packs for the attention lo-pass:
               q-pack = (qh/64, qres*64), k-pack = (kres*64, kh/64).
      phase 2: per (batch, head): fp16 S~=QK^T in [q,k] layout for row-max
               (diagonal causal mask folded in as an identity x mask-const
               matmul on the PE); exact S^T in [k,q] via fp16 hi matmul
               (with folded -max bias row) + one fp8 DoubleRow lo matmul;
               exp on ScalarE; PV + row-sums via fp32r matmul with a 0.25
               column appended to V (so the normalizer is 1/(4l)).
      phase 3: o_proj via 2-level fp8 DoubleRow: AO0=fp8(ao/4),
               AO1=fp8(ao/4-AO0), weights 4*ternary (exact in fp8); fp16
               partial outputs DMA'd per batch so they overlap attention.
  - host: sum the 8 fp16 partial outputs in fp32 (row-split "all-reduce").
"""

import sys

sys.path.insert(0, "/opt/trn_rl_repo")

import numpy as np
import ml_dtypes

B = 2
S = 2048
D = 2048
NCORES = 8
HEADS_PER_CORE = 4
HD = 64
QROWS = HEADS_PER_CORE * HD  # 256
TT = 512  # token tile
MASK_NEG = -30000.0
XL_SCALE = 512.0     # x residual fp8 scale (weights get 1/512)
PK_SCALE = 64.0      # S lo-pass pack scale (carrier /64, residual x64)

E4 = ml_dtypes.float8_e4m3

_CACHE = {}


def _build_program(b=B, s=S, d=D):
    import concourse.bacc as bacc
    import concourse.tile as tile
    import concourse.mybir as mybir
    from concourse import masks
    from contextlib import ExitStack

    f32 = mybir.dt.float32
    f32r = mybir.dt.float32r
    f16 = mybir.dt.float16
    f8 = mybir.dt.float8e4
    Alu = mybir.AluOpType
    Act = mybir.ActivationFunctionType
    DR = mybir.MatmulPerfMode.DoubleRow

    tokens = b * s
    n_tt = tokens // TT          # token tiles
    tt_per_b = s // TT
    n_dc = d // 128              # contraction chunks for projections
    n_qt = s // TT               # 512-wide q tiles per batch
    n_qc = s // 128              # 128-wide q chunks per batch (max pass)
    n_mt = d // 128              # output row tiles for o_proj
    n_oc = QROWS // 128          # o_proj contraction chunks (2)
    sub = TT // 128              # 128-sub-blocks per 512 tile (4)

    nc = bacc.Bacc("TRN2", target_bir_lowering=False, debug=False,
                   num_devices=NCORES)

    xh_d = nc.dram_tensor("xh", [d, tokens], f16, kind="ExternalInput").ap()
    xl_d = nc.dram_tensor("xl", [d, tokens], f8, kind="ExternalInput").ap()
    wqh_d = nc.dram_tensor("wq_hi", [d, QROWS], f16, kind="ExternalInput").ap()
    wql_d = nc.dram_tensor("wq_lo", [d, QROWS], f8, kind="ExternalInput").ap()
    wkh_d = nc.dram_tensor("wkv_hi", [d, 128], f16, kind="ExternalInput").ap()
    wkl_d = nc.dram_tensor("wkv_lo", [d, 128], f8, kind="ExternalInput").ap()
    wo_d = nc.dram_tensor("wo8", [QROWS, d], f8, kind="ExternalInput").ap()
    out_d = nc.dram_tensor("out", [d, tokens], f16, kind="ExternalOutput").ap()

    with tile.TileContext(nc) as tc, ExitStack() as top:
        constp = top.enter_context(tc.tile_pool(name="const", bufs=1))
        wpool = top.enter_context(tc.tile_pool(name="wts", bufs=1))
        pp = top.enter_context(tc.tile_pool(name="persist", bufs=1))

        # --- constants -------------------------------------------------
        # maskKQ[p,q] = MASK_NEG where p > q (keep k<=q in [k,q] layout)
        maskKQ = constp.tile([128, 128], f16, tag="maskKQ")
        nc.gpsimd.memset(maskKQ[:], 0.0)
        nc.gpsimd.affine_select(
            out=maskKQ[:], in_=maskKQ[:], compare_op=Alu.is_ge, fill=MASK_NEG,
            base=0, pattern=[[1, 128]], channel_multiplier=-1)
        # maskQK[q,k] = MASK_NEG where k > q ([q,k] layout)
        maskQK = constp.tile([128, 128], f16, tag="maskQK")
        nc.gpsimd.memset(maskQK[:], 0.0)
        nc.gpsimd.affine_select(
            out=maskQK[:], in_=maskQK[:], compare_op=Alu.is_ge, fill=MASK_NEG,
            base=0, pattern=[[-1, 128]], channel_multiplier=1)
        identM = constp.tile([128, 128], f16, tag="identM")
        masks.make_identity(nc, identM[:])
        ident = constp.tile([128, 128], f32, tag="ident")
        masks.make_identity(nc, ident[:])
        # bc row value: bcs = 0.0625 * (4/l) = 0.25/l, so aof = PV*bcs = attn/4
        onesc = constp.tile([65, HD], f32r, tag="onesc")
        nc.gpsimd.memset(onesc[:], 0.0625)

        # --- weights (DMA order tuned so tile-0 compute starts early) ---
        wq16 = wpool.tile([128, n_dc * QROWS], f16, tag="wq16", name="wq16")
        nc.sync.dma_start(
            out=wq16[:].rearrange("p (c n) -> p c n", n=QROWS)[:, :, 0:128],
            in_=wqh_d.rearrange("(c p) n -> p c n", p=128)[:, :, 0:128])
        wq8 = wpool.tile([128, n_dc, QROWS], f8, tag="wq8", name="wq8")
        wkv16 = wpool.tile([128, n_dc * 128], f16, tag="wkv16", name="wkv16")
        wkv8 = wpool.tile([128, n_dc, 128], f8, tag="wkv8", name="wkv8")

        def load_weights_rest():
            nc.sync.dma_start(
                out=wq16[:].rearrange(
                    "p (c n) -> p c n", n=QROWS)[:, :, 128:QROWS],
                in_=wqh_d.rearrange("(c p) n -> p c n", p=128)[:, :,
                                                              128:QROWS])
            nc.sync.dma_start(
                out=wq8[:], in_=wql_d.rearrange("(c p) n -> p c n", p=128))
            nc.sync.dma_start(
                out=wkv16[:].rearrange("p (c n) -> p c n", n=128),
                in_=wkh_d.rearrange("(c p) n -> p c n", p=128))
            nc.sync.dma_start(
                out=wkv8[:], in_=wkl_d.rearrange("(c p) n -> p c n", p=128))

        # o_proj weights: [128, 2(level), d] per contraction chunk; both
        # levels are the same 4*ternary data. Loaded at end of phase 1.
        wo8 = [wpool.tile([128, 2, d], f8, tag=f"wo8_{c}", name=f"wo8_{c}")
               for c in range(n_oc)]

        def load_wo8():
            for c in range(n_oc):
                for lv in range(2):
                    nc.sync.dma_start(
                        out=wo8[c][:, lv, :],
                        in_=wo_d[c * 128:(c + 1) * 128, :])

        # --- persistent activations -----------------------------------
        # qA[h]: rows 0:64 = fp16(q/8), row 64 = m~ bias (max pass)
        # qP[h]: fp8 pack [64, 2, tokens] = (qh/64, qres*64)
        # khb:   rows 0:64 = fp16(k), row 64 = -1
        # kP:    fp8 pack [64, 2, tokens] = (kres*64, kh/64)
        # vhat:  [128, chunk*65]: cols 0:64 of chunk = v, col 64 = 0.25
        qA = [pp.tile([65, tokens], f16, tag=f"qA{h}", name=f"qA{h}")
              for h in range(HEADS_PER_CORE)]
        qP = [pp.tile([64, 2, tokens], f8, tag=f"qP{h}", name=f"qP{h}")
              for h in range(HEADS_PER_CORE)]
        khb = pp.tile([65, tokens], f16, tag="khb")
        kP = pp.tile([64, 2, tokens], f8, tag="kP")
        n_ch = tokens // 128
        vhat = pp.tile([128, n_ch * 65], f32r, tag="vhat")
        nc.gpsimd.memset(vhat[:], 0.25)
        nc.gpsimd.memset(khb[64:65, :], -1.0)

        with ExitStack() as ph:
            mp = ph.enter_context(tc.tile_pool(name="mp", bufs=2))
            ps1 = ph.enter_context(
                tc.tile_pool(name="ps1", bufs=3, space="PSUM"))
            psst = ph.enter_context(
                tc.tile_pool(name="psst", bufs=2, space="PSUM"))
            psav = ph.enter_context(
                tc.tile_pool(name="psav", bufs=2, space="PSUM"))
            psbc = ph.enter_context(
                tc.tile_pool(name="psbc", bufs=1, space="PSUM"))

            # ---------- S~ max-estimate pass, as schedulable blocks ------
            mstate = {}
            mbp = ph.enter_context(tc.tile_pool(name="mbp", bufs=8))

            def s_block(bb, h, qc):
                boff = bb * s
                if qc == 0:
                    mstate[(bb, h)] = mbp.tile([128, n_qc], f32, tag="mbuf",
                                               name="mbuf")
                mbuf = mstate[(bb, h)]
                qsl = slice(boff + qc * 128, boff + qc * 128 + 128)
                ntk = qc // sub + 1
                mtmp = mp.tile([128, 8], f32, tag="mtmp")
                for kt in range(ntk):
                    w = min(TT, (qc + 1) * 128 - kt * TT)
                    st = psst.tile([128, TT], f32, tag="st")
                    nc.tensor.matmul(
                        st[:, 0:w],
                        lhsT=qA[h][0:64, qsl],
                        rhs=khb[0:64, boff + kt * TT:boff + kt * TT + w],
                        start=True, stop=(kt != ntk - 1))
                    if kt == ntk - 1:  # diagonal block is last 128 cols
                        nc.tensor.matmul(
                            st[:, w - 128:w], lhsT=identM[:], rhs=maskQK[:],
                            start=False, stop=True, skip_group_check=True)
                    nc.vector.tensor_reduce(
                        mtmp[:, kt:kt + 1], st[:, 0:w],
                        axis=mybir.AxisListType.X, op=Alu.max)
                nc.vector.tensor_reduce(
                    mbuf[:, qc:qc + 1], mtmp[:, 0:ntk],
                    axis=mybir.AxisListType.X, op=Alu.max)

            def s_final(bb, h):
                boff = bb * s
                mbuf = mstate.pop((bb, h))
                mps = psst.tile([128, TT], f32, tag="st")
                nc.tensor.transpose(mps[0:n_qc, 0:128], mbuf[:, 0:n_qc],
                                    ident[:, 0:128])
                mrow = mp.tile([n_qc, 128], f32, tag="mrow")
                nc.vector.tensor_copy(mrow[:], mps[0:n_qc, 0:128])
                nc.gpsimd.dma_start(
                    out=qA[h][64:65, boff:boff + s].rearrange(
                        "o (c t) -> o c t", t=128),
                    in_=mrow[:])

            # ================= phase 1: projections ====================
            ph1 = ExitStack()
            xp = ph1.enter_context(tc.tile_pool(name="xp", bufs=2))
            x8p = ph1.enter_context(tc.tile_pool(name="x8p", bufs=2))
            sp1 = ph1.enter_context(tc.tile_pool(name="sp1", bufs=3))
            for tt in range(n_tt):
                tcols = slice(tt * TT, (tt + 1) * TT)
                xt = xp.tile([128, n_dc * TT], f16, tag="x", name="xtile")
                nc.sync.dma_start(
                    out=xt[:].rearrange("p (c t) -> p c t", t=TT),
                    in_=xh_d.rearrange("(c p) t -> p c t", p=128)[:, :, tcols])
                x8 = x8p.tile([128, n_dc, TT], f8, tag="x8", name="x8tile")
                nc.sync.dma_start(
                    out=x8[:],
                    in_=xl_d.rearrange("(c p) t -> p c t", p=128)[:, :, tcols])
                if tt == 0:
                    load_weights_rest()

                def proj(w16, w8, mcol, mwid, ps):
                    for c in range(n_dc):
                        nc.tensor.matmul(
                            ps[:],
                            lhsT=w16[:, c * mwid + mcol:c * mwid + mcol + 128],
                            rhs=xt[:, c * TT:(c + 1) * TT],
                            start=(c == 0), stop=False)
                    for half in range(2):
                        hs = slice(half * 256, half * 256 + 256)
                        for cp in range(n_dc // 2):
                            nc.tensor.matmul(
                                ps[:, hs],
                                lhsT=w8[:, 2 * cp:2 * cp + 2,
                                        mcol:mcol + 128],
                                rhs=x8[:, 2 * cp:2 * cp + 2, hs],
                                start=False,
                                stop=(half == 1 and cp == n_dc // 2 - 1),
                                perf_mode=DR, skip_group_check=True)

                for m in range(QROWS // 128):
                    ps = ps1.tile([128, TT], f32, tag="ps")
                    proj(wq16, wq8, m * 128, QROWS, ps)
                    for i in range(2):
                        h = 2 * m + i
                        rows = slice(i * 64, i * 64 + 64)
                        # qA = fp16(q/8)
                        nc.scalar.activation(qA[h][0:64, tcols], ps[rows, :],
                                             Act.Copy, scale=0.125)
                        # res = q/8 - qA (fp32)
                        res = sp1.tile([64, TT], f32, tag="qres")
                        nc.vector.scalar_tensor_tensor(
                            res[:], in0=ps[rows, :], scalar=0.125,
                            in1=qA[h][0:64, tcols],
                            op0=Alu.mult, op1=Alu.subtract)
                        # fp8 pack (SBUF->SBUF on gpsimd)
                        nc.gpsimd.tensor_scalar_mul(
                            qP[h][:, 0, tcols], qA[h][0:64, tcols],
                            1.0 / PK_SCALE)
                        nc.gpsimd.tensor_scalar_mul(
                            qP[h][:, 1, tcols], res[:], PK_SCALE)

                ps = ps1.tile([128, TT], f32, tag="ps")
                proj(wkv16, wkv8, 0, 128, ps)
                nc.scalar.copy(khb[0:64, tcols], ps[0:64, :])
                res = sp1.tile([64, TT], f32, tag="qres")
                nc.vector.scalar_tensor_tensor(
                    res[:], in0=ps[0:64, :], scalar=1.0,
                    in1=khb[0:64, tcols], op0=Alu.mult, op1=Alu.subtract)
                nc.gpsimd.tensor_scalar_mul(
                    kP[:, 0, tcols], res[:], PK_SCALE)
                nc.gpsimd.tensor_scalar_mul(
                    kP[:, 1, tcols], khb[0:64, tcols], 1.0 / PK_SCALE)
                vtmp = sp1.tile([64, TT], f32, tag="vtmp")
                nc.scalar.copy(vtmp[:], ps[64:128, :])
                for j in range(sub):
                    ptr = psst.tile([128, TT], f32, tag="st")
                    nc.tensor.transpose(ptr[0:128, 0:64],
                                        vtmp[:, j * 128:(j + 1) * 128],
                                        ident[0:64, 0:64])
                    ch = tt * sub + j
                    nc.scalar.copy(vhat[:, ch * 65:ch * 65 + 64],
                                   ptr[0:128, 0:64])
                # batch-0 S~ blocks for the q-chunks this tile enabled
                bb, ltt = tt // tt_per_b, tt % tt_per_b
                if bb == 0:
                    for h in range(HEADS_PER_CORE):
                        for qc in range(ltt * sub, (ltt + 1) * sub):
                            s_block(bb, h, qc)
                    if ltt == tt_per_b - 1:
                        for h in range(HEADS_PER_CORE):
                            s_final(bb, h)

            ph1.close()
            # ============ phase 2 + per-batch o_proj ====================
            load_wo8()
            aop = ph.enter_context(tc.tile_pool(name="aop", bufs=1))
            ptp = ph.enter_context(tc.tile_pool(name="ptp", bufs=4))
            outp = ph.enter_context(tc.tile_pool(name="outp", bufs=2))
            # ao[c]: fp8 pack [128, 2(level), tokens]
            ao = [aop.tile([128, 2, tokens], f8, tag=f"ao{i}", name=f"ao{i}")
                  for i in range(n_oc)]

            def av(pav, pt, lo, w, bb, kc, nchunks):
                ch = bb * (s // 128) + kc
                nc.tensor.matmul(
                    pav[:, lo:lo + w], lhsT=vhat[:, ch * 65:ch * 65 + 65],
                    rhs=pt[:, lo:lo + w],
                    start=(kc == 0), stop=(kc == nchunks - 1),
                    skip_group_check=True)

            b1q = []
            for h in range(HEADS_PER_CORE):
                for qc in range(n_qc):
                    b1q.append(lambda h=h, qc=qc: s_block(1, h, qc))
                b1q.append(lambda h=h: s_final(1, h))
            # weight S~ pacing by main-slot size (qt+1 chunks of work)
            wsum = HEADS_PER_CORE * n_qt * (n_qt + 1) // 2
            bcum, acc = [], 0.0
            for h in range(HEADS_PER_CORE):
                for qt in range(n_qt):
                    acc += (qt + 1) * len(b1q) / wsum
                    bcum.append(min(int(round(acc)), len(b1q)))
            bcum[-1] = len(b1q)

            def oproj_part(bb, m, qt0, nqt):
                boff = bb * s
                osb = outp.tile([128, nqt * TT], f16, tag="ot", name="osb")
                for i in range(nqt):
                    for half in range(2):
                        po = ps1.tile([128, TT], f32, tag="ps")
                        qsl = slice(boff + (qt0 + i) * TT + half * 256,
                                    boff + (qt0 + i) * TT + half * 256 + 256)
                        for ci in range(n_oc):
                            nc.tensor.matmul(
                                po[:, 0:256],
                                lhsT=wo8[ci][:, :, m * 128:m * 128 + 128],
                                rhs=ao[ci][:, :, qsl],
                                start=(ci == 0), stop=(ci == n_oc - 1),
                                perf_mode=DR)
                        dst = osb[:, i * TT + half * 256:
                                  i * TT + half * 256 + 256]
                        if (m + i + half) % 2 == 0:
                            nc.vector.tensor_copy(dst, po[:, 0:256])
                        else:
                            nc.scalar.copy(dst, po[:, 0:256])
                nc.sync.dma_start(
                    out=out_d[m * 128:(m + 1) * 128,
                              boff + qt0 * TT:boff + (qt0 + nqt) * TT],
                    in_=osb[:])

            def attn_slot(bb, h, qt):
                boff = bb * s
                qlo = boff + qt * TT
                pav = psav.tile([65, TT], f32, tag="pav")
                nchunks = (qt + 1) * sub
                prev = None
                for kc in range(nchunks):
                    ksl = slice(boff + kc * 128, boff + kc * 128 + 128)
                    j = kc - qt * sub
                    lo = max(j, 0) * 128  # cols < lo fully masked
                    w = TT - lo
                    s2 = ps1.tile([128, TT], f32, tag="ps")
                    nc.tensor.matmul(
                        s2[:, lo:lo + w], lhsT=khb[:, ksl],
                        rhs=qA[h][:, qlo + lo:qlo + TT],
                        start=True, stop=False)
                    # fp8 DoubleRow lo-pass (<=256-wide halves)
                    nhalf = (w + 255) // 256
                    for hf in range(nhalf):
                        hlo = lo + hf * 256
                        hw = min(256, TT - hlo)
                        nc.tensor.matmul(
                            s2[:, hlo:hlo + hw],
                            lhsT=kP[:, :, ksl],
                            rhs=qP[h][:, :, qlo + hlo:qlo + hlo + hw],
                            start=False,
                            stop=(j < 0 and hf == nhalf - 1),
                            perf_mode=DR, skip_group_check=True)
                    if j >= 0:
                        nc.tensor.matmul(
                            s2[:, lo:lo + 128], lhsT=identM[:],
                            rhs=maskKQ[:], start=False, stop=True,
                            skip_group_check=True)
                    pt = ptp.tile([128, TT], f32r, tag="pt")
                    nc.scalar.activation(pt[:, lo:lo + w],
                                         s2[:, lo:lo + w], Act.Exp)
                    if prev is not None:
                        av(pav, prev[0], prev[1], prev[2], bb,
                           prev[3], nchunks)
                    prev = (pt, lo, w, kc)
                av(pav, prev[0], prev[1], prev[2], bb, prev[3], nchunks)

                # pav row 64 = l/4; rec = 4/l; bcs = 0.0625*4/l
                # = 0.25/l, so aof = PV * bcs = attn/4 (fp8-safe)
                rec = mp.tile([65, TT], f32r, tag="rec")
                with nc.allow_low_precision(
                        reason="1/l broadcast feeds fp32r matmul"):
                    nc.vector.reciprocal(rec[64:65, :], pav[64:65, :])
                bc = psbc.tile([64, TT], f32, tag="bc")
                nc.tensor.matmul(
                    bc[:], lhsT=onesc[64:65, 0:HD],
                    rhs=rec[64:65, :], start=True, stop=True)
                bcs = mp.tile([64, TT], f32, tag="bcs")
                nc.scalar.copy(bcs[:], bc[:])
                # AO0 = fp8(aof), AO1 = fp8(aof - AO0); wo8 = 4*tern
                # aof half matches ao's base partition (SB+SB ops
                # require equal base partitions)
                rows = slice((h % 2) * 64, (h % 2) * 64 + 64)
                aof = mp.tile([128, TT], f32, tag="aof")
                nc.vector.tensor_tensor(
                    aof[rows, :], pav[0:64, :], bcs[:], op=Alu.mult)
                c = h // 2
                nc.gpsimd.tensor_copy(
                    ao[c][rows, 0, qlo:qlo + TT], aof[rows, :])
                nc.gpsimd.scalar_tensor_tensor(
                    ao[c][rows, 1, qlo:qlo + TT],
                    in0=ao[c][rows, 0, qlo:qlo + TT], scalar=-1.0,
                    in1=aof[rows, :], op0=Alu.mult, op1=Alu.add)

            # batch 0: h-major, batch-1 S~ ridesharing in slot gaps
            slot = 0
            for h in range(HEADS_PER_CORE):
                for qt in range(n_qt):
                    attn_slot(0, h, qt)
                    lo_i = bcum[slot - 1] if slot else 0
                    for fn in b1q[lo_i:bcum[slot]]:
                        fn()
                    slot += 1
            # batch 1: qt-major so its o_proj can interleave per q-tile;
            # batch-0 o_proj spread over the slots
            per = (n_mt + HEADS_PER_CORE * n_qt - 1) \
                // (HEADS_PER_CORE * n_qt)
            for qt in range(n_qt):
                for h in range(HEADS_PER_CORE):
                    attn_slot(1, h, qt)
                    si = qt * HEADS_PER_CORE + h
                    for m in range(si * per, min((si + 1) * per, n_mt)):
                        oproj_part(0, m, 0, n_qt)
                for m in range(n_mt):
                    oproj_part(1, m, qt, 1)

    nc.compile()
    return nc


def _ternarize(w):
    w = np.asarray(w, np.float32)
    scale = max(np.abs(w).mean(), 1e-6)
    return ((w > 0.05 * scale).astype(np.float32)
            - (w < -0.05 * scale).astype(np.float32))


def kernel(x, wq, wk, wv, wo):
    from concourse.bass_utils import run_bass_kernel_spmd

    if "nc" not in _CACHE:
        _CACHE["nc"] = _build_program()
    nc = _CACHE["nc"]

    tq = _ternarize(wq)
    tk = _ternarize(wk)
    tv = _ternarize(wv)
    to = _ternarize(wo)

    xT = np.ascontiguousarray(np.asarray(x, np.float32).reshape(B * S, D).T)
    xh = xT.astype(np.float16)
    xl = ((xT - xh.astype(np.float32)) * XL_SCALE).astype(E4)

    in_maps = []
    for c in range(NCORES):
        qsl = slice(c * QROWS, (c + 1) * QROWS)
        ksl = slice(c * HD, (c + 1) * HD)
        wkv = np.concatenate([tk[ksl], tv[ksl]], axis=0)  # [128, D]
        wqT = np.ascontiguousarray(tq[qsl].T)
        wkvT = np.ascontiguousarray(wkv.T)
        in_maps.append({
            "xh": xh, "xl": xl,
            "wq_hi": wqT.astype(np.float16),
            "wq_lo": (wqT / XL_SCALE).astype(E4),
            "wkv_hi": wkvT.astype(np.float16),
            "wkv_lo": (wkvT / XL_SCALE).astype(E4),
            "wo8": np.ascontiguousarray(to[:, qsl].T * 4.0).astype(E4),
        })

    res = run_bass_kernel_spmd(nc, in_maps, list(range(NCORES)))
    total = res.results[0]["out"].astype(np.float32)
    for c in range(1, NCORES):
        total = total + res.results[c]["out"].astype(np.float32)
    return np.ascontiguousarray(total.T).reshape(B, S, D).astype(np.float32)


# revision 16
# speedup vs baseline: 1.2718x; 1.1229x over previous
"""GQA causal attention (ternary weights) on 8 TRN2 NeuronCores.

Strategy (tensor-parallel over heads, per sharding hint):
  - core c owns Q heads [4c, 4c+4) and KV head c.
  - host: ternarize weights; split x into fp16 hi + fp8(e4m3, x512) residual;
    fp8 weight copies are ternary/512 (exact: 2^-9 is an e4m3 subnormal) so
    the fp8 DoubleRow lo-pass lands at natural scale in the same PSUM group
    as the fp16 hi-pass.
  - device per core:
      phase 1: q/k/v projections: fp16 hi matmuls + fp8 DoubleRow lo matmuls
               (2 contraction chunks per instruction, half cost). Activations
               are stored as fp16 (qA rows 0:64 = q/8, khb rows 0:64 = k,
               row 64 = bias) plus fp8 packs for the attention lo-pass:
               q-pack = (qh/64, qres*64), k-pack = (kres*64, kh/64).
      phase 2: per (batch, head): fp16 S~=QK^T in [q,k] layout for row-max
               (diagonal causal mask folded in as an identity x mask-const
               matmul on the PE); exact S^T in [k,q] via fp16 hi matmul
               (with folded -max bias row) + one fp8 DoubleRow lo matmul;
               exp on ScalarE; PV + row-sums via fp32r matmul with a 0.25
               column appended to V (so the normalizer is 1/(4l)).
      phase 3: o_proj via 2-level fp8 DoubleRow: AO0=fp8(ao/4),
               AO1=fp8(ao/4-AO0), weights 4*ternary (exact in fp8); fp16
               partial outputs DMA'd per batch so they overlap attention.
  - host: sum the 8 fp16 partial outputs in fp32 (row-split "all-reduce").
"""

import sys

sys.path.insert(0, "/opt/trn_rl_repo")

import numpy as np
import ml_dtypes

B = 2
S = 2048
D = 2048
NCORES = 8
HEADS_PER_CORE = 4
HD = 64
QROWS = HEADS_PER_CORE * HD  # 256
TT = 512  # token tile
MASK_NEG = -30000.0
XL_SCALE = 512.0     # x residual fp8 scale (weights get 1/512)
PK_SCALE = 64.0      # S lo-pass pack scale (carrier /64, residual x64)

E4 = ml_dtypes.float8_e4m3

_CACHE = {}


def _build_program(b=B, s=S, d=D):
    import concourse.bacc as bacc
    import concourse.tile as tile
    import concourse.mybir as mybir
    from concourse import masks
    from contextlib import ExitStack

    f32 = mybir.dt.float32
    f32r = mybir.dt.float32r
    f16 = mybir.dt.float16
    f8 = mybir.dt.float8e4
    Alu = mybir.AluOpType
    Act = mybir.ActivationFunctionType
    DR = mybir.MatmulPerfMode.DoubleRow

    tokens = b * s
    n_tt = tokens // TT          # token tiles
    tt_per_b = s // TT
    n_dc = d // 128              # contraction chunks for projections
    n_qt = s // TT               # 512-wide q tiles per batch
    n_qc = s // 128              # 128-wide q chunks per batch (max pass)
    n_mt = d // 128              # output row tiles for o_proj
    n_oc = QROWS // 128          # o_proj contraction chunks (2)
    sub = TT // 128              # 128-sub-blocks per 512 tile (4)

    nc = bacc.Bacc("TRN2", target_bir_lowering=False, debug=False,
                   num_devices=NCORES)

    xh_d = nc.dram_tensor("xh", [d, tokens], f16, kind="ExternalInput").ap()
    xl_d = nc.dram_tensor("xl", [d, tokens], f8, kind="ExternalInput").ap()
    wqh_d = nc.dram_tensor("wq_hi", [d, QROWS], f16, kind="ExternalInput").ap()
    wql_d = nc.dram_tensor("wq_lo", [d, QROWS], f8, kind="ExternalInput").ap()
    wkh_d = nc.dram_tensor("wkv_hi", [d, 128], f16, kind="ExternalInput").ap()
    wkl_d = nc.dram_tensor("wkv_lo", [d, 128], f8, kind="ExternalInput").ap()
    wo_d = nc.dram_tensor("wo8", [QROWS, d], f8, kind="ExternalInput").ap()
    out_d = nc.dram_tensor("out", [d, tokens], f16, kind="ExternalOutput").ap()

    with tile.TileContext(nc) as tc, ExitStack() as top:
        constp = top.enter_context(tc.tile_pool(name="const", bufs=1))
        wpool = top.enter_context(tc.tile_pool(name="wts", bufs=1))
        pp = top.enter_context(tc.tile_pool(name="persist", bufs=1))

        # --- constants -------------------------------------------------
        # maskKQ[p,q] = MASK_NEG where p > q (keep k<=q in [k,q] layout)
        maskKQ = constp.tile([128, 128], f16, tag="maskKQ")
        nc.gpsimd.memset(maskKQ[:], 0.0)
        nc.gpsimd.affine_select(
            out=maskKQ[:], in_=maskKQ[:], compare_op=Alu.is_ge, fill=MASK_NEG,
            base=0, pattern=[[1, 128]], channel_multiplier=-1)
        # maskQK[q,k] = MASK_NEG where k > q ([q,k] layout)
        maskQK = constp.tile([128, 128], f16, tag="maskQK")
        nc.gpsimd.memset(maskQK[:], 0.0)
        nc.gpsimd.affine_select(
            out=maskQK[:], in_=maskQK[:], compare_op=Alu.is_ge, fill=MASK_NEG,
            base=0, pattern=[[-1, 128]], channel_multiplier=1)
        identM = constp.tile([128, 128], f16, tag="identM")
        masks.make_identity(nc, identM[:])
        ident = constp.tile([128, 128], f32, tag="ident")
        masks.make_identity(nc, ident[:])
        # bc row value: bcs = 0.0625 * (4/l) = 0.25/l, so aof = PV*bcs = attn/4
        onesc = constp.tile([65, HD], f32r, tag="onesc")
        nc.gpsimd.memset(onesc[:], 0.0625)

        # --- weights (DMA order tuned so tile-0 compute starts early) ---
        wq16 = wpool.tile([128, n_dc * QROWS], f16, tag="wq16", name="wq16")
        nc.sync.dma_start(
            out=wq16[:].rearrange("p (c n) -> p c n", n=QROWS)[:, :, 0:128],
            in_=wqh_d.rearrange("(c p) n -> p c n", p=128)[:, :, 0:128])
        wq8 = wpool.tile([128, n_dc, QROWS], f8, tag="wq8", name="wq8")
        wkv16 = wpool.tile([128, n_dc * 128], f16, tag="wkv16", name="wkv16")
        wkv8 = wpool.tile([128, n_dc, 128], f8, tag="wkv8", name="wkv8")

        def load_weights_rest():
            nc.sync.dma_start(
                out=wq16[:].rearrange(
                    "p (c n) -> p c n", n=QROWS)[:, :, 128:QROWS],
                in_=wqh_d.rearrange("(c p) n -> p c n", p=128)[:, :,
                                                              128:QROWS])
            nc.sync.dma_start(
                out=wq8[:], in_=wql_d.rearrange("(c p) n -> p c n", p=128))
            nc.sync.dma_start(
                out=wkv16[:].rearrange("p (c n) -> p c n", n=128),
                in_=wkh_d.rearrange("(c p) n -> p c n", p=128))
            nc.sync.dma_start(
                out=wkv8[:], in_=wkl_d.rearrange("(c p) n -> p c n", p=128))

        # o_proj weights: [128, 2(level), d] per contraction chunk; both
        # levels are the same 4*ternary data. Loaded at end of phase 1.
        wo8 = [wpool.tile([128, 2, d], f8, tag=f"wo8_{c}", name=f"wo8_{c}")
               for c in range(n_oc)]

        def load_wo8():
            for c in range(n_oc):
                for lv in range(2):
                    nc.sync.dma_start(
                        out=wo8[c][:, lv, :],
                        in_=wo_d[c * 128:(c + 1) * 128, :])

        # --- persistent activations -----------------------------------
        # qA[h]: rows 0:64 = fp16(q/8), row 64 = m~ bias (max pass)
        # qP[h]: fp8 pack [64, 2, tokens] = (qh/64, qres*64)
        # khb:   rows 0:64 = fp16(k), row 64 = -1
        # kP:    fp8 pack [64, 2, tokens] = (kres*64, kh/64)
        # vhat:  [128, chunk*65]: cols 0:64 of chunk = v, col 64 = 0.25
        qA = [pp.tile([65, tokens], f16, tag=f"qA{h}", name=f"qA{h}")
              for h in range(HEADS_PER_CORE)]
        qP = [pp.tile([64, 2, tokens], f8, tag=f"qP{h}", name=f"qP{h}")
              for h in range(HEADS_PER_CORE)]
        khb = pp.tile([65, tokens], f16, tag="khb")
        kP = pp.tile([64, 2, tokens], f8, tag="kP")
        n_ch = tokens // 128
        vhat = pp.tile([128, n_ch * 65], f32r, tag="vhat")
        nc.gpsimd.memset(vhat[:], 0.25)
        nc.gpsimd.memset(khb[64:65, :], -1.0)

        with ExitStack() as ph:
            mp = ph.enter_context(tc.tile_pool(name="mp", bufs=2))
            ps1 = ph.enter_context(
                tc.tile_pool(name="ps1", bufs=3, space="PSUM"))
            psst = ph.enter_context(
                tc.tile_pool(name="psst", bufs=2, space="PSUM"))
            psav = ph.enter_context(
                tc.tile_pool(name="psav", bufs=2, space="PSUM"))
            psbc = ph.enter_context(
                tc.tile_pool(name="psbc", bufs=1, space="PSUM"))

            # ---------- S~ max-estimate pass, as schedulable blocks ------
            mstate = {}
            mbp = ph.enter_context(tc.tile_pool(name="mbp", bufs=8))

            def s_block(bb, h, qc):
                boff = bb * s
                if qc == 0:
                    mstate[(bb, h)] = mbp.tile([128, n_qc], f32, tag="mbuf",
                                               name="mbuf")
                mbuf = mstate[(bb, h)]
                qsl = slice(boff + qc * 128, boff + qc * 128 + 128)
                ntk = qc // sub + 1
                mtmp = mp.tile([128, 8], f32, tag="mtmp")
                for kt in range(ntk):
                    w = min(TT, (qc + 1) * 128 - kt * TT)
                    st = psst.tile([128, TT], f32, tag="st")
                    nc.tensor.matmul(
                        st[:, 0:w],
                        lhsT=qA[h][0:64, qsl],
                        rhs=khb[0:64, boff + kt * TT:boff + kt * TT + w],
                        start=True, stop=(kt != ntk - 1))
                    if kt == ntk - 1:  # diagonal block is last 128 cols
                        nc.tensor.matmul(
                            st[:, w - 128:w], lhsT=identM[:], rhs=maskQK[:],
                            start=False, stop=True, skip_group_check=True)
                    nc.vector.tensor_reduce(
                        mtmp[:, kt:kt + 1], st[:, 0:w],
                        axis=mybir.AxisListType.X, op=Alu.max)
                nc.vector.tensor_reduce(
                    mbuf[:, qc:qc + 1], mtmp[:, 0:ntk],
                    axis=mybir.AxisListType.X, op=Alu.max)

            def s_final(bb, h):
                boff = bb * s
                mbuf = mstate.pop((bb, h))
                mps = psst.tile([128, TT], f32, tag="st")
                nc.tensor.transpose(mps[0:n_qc, 0:128], mbuf[:, 0:n_qc],
                                    ident[:, 0:128])
                mrow = mp.tile([n_qc, 128], f32, tag="mrow")
                nc.vector.tensor_copy(mrow[:], mps[0:n_qc, 0:128])
                nc.gpsimd.dma_start(
                    out=qA[h][64:65, boff:boff + s].rearrange(
                        "o (c t) -> o c t", t=128),
                    in_=mrow[:])

            # ================= phase 1: projections ====================
            ph1 = ExitStack()
            xp = ph1.enter_context(tc.tile_pool(name="xp", bufs=2))
            x8p = ph1.enter_context(tc.tile_pool(name="x8p", bufs=2))
            sp1 = ph1.enter_context(tc.tile_pool(name="sp1", bufs=3))
            for tt in range(n_tt):
                tcols = slice(tt * TT, (tt + 1) * TT)
                xt = xp.tile([128, n_dc * TT], f16, tag="x", name="xtile")
                nc.sync.dma_start(
                    out=xt[:].rearrange("p (c t) -> p c t", t=TT),
                    in_=xh_d.rearrange("(c p) t -> p c t", p=128)[:, :, tcols])
                x8 = x8p.tile([128, n_dc, TT], f8, tag="x8", name="x8tile")
                nc.sync.dma_start(
                    out=x8[:],
                    in_=xl_d.rearrange("(c p) t -> p c t", p=128)[:, :, tcols])
                if tt == 0:
                    load_weights_rest()

                def proj(w16, w8, mcol, mwid, ps):
                    for c in range(n_dc):
                        nc.tensor.matmul(
                            ps[:],
                            lhsT=w16[:, c * mwid + mcol:c * mwid + mcol + 128],
                            rhs=xt[:, c * TT:(c + 1) * TT],
                            start=(c == 0), stop=False)
                    for half in range(2):
                        hs = slice(half * 256, half * 256 + 256)
                        for cp in range(n_dc // 2):
                            nc.tensor.matmul(
                                ps[:, hs],
                                lhsT=w8[:, 2 * cp:2 * cp + 2,
                                        mcol:mcol + 128],
                                rhs=x8[:, 2 * cp:2 * cp + 2, hs],
                                start=False,
                                stop=(half == 1 and cp == n_dc // 2 - 1),
                                perf_mode=DR, skip_group_check=True)

                for m in range(QROWS // 128):
                    ps = ps1.tile([128, TT], f32, tag="ps")
                    proj(wq16, wq8, m * 128, QROWS, ps)
                    for i in range(2):
                        h = 2 * m + i
                        rows = slice(i * 64, i * 64 + 64)
                        # qA = fp16(q/8)
                        nc.scalar.activation(qA[h][0:64, tcols], ps[rows, :],
                                             Act.Copy, scale=0.125)
                        # res = q/8 - qA (fp32)
                        res = sp1.tile([64, TT], f32, tag="qres")
                        nc.vector.scalar_tensor_tensor(
                            res[:], in0=ps[rows, :], scalar=0.125,
                            in1=qA[h][0:64, tcols],
                            op0=Alu.mult, op1=Alu.subtract)
                        # fp8 pack (SBUF->SBUF on gpsimd)
                        nc.gpsimd.tensor_scalar_mul(
                            qP[h][:, 0, tcols], qA[h][0:64, tcols],
                            1.0 / PK_SCALE)
                        nc.gpsimd.tensor_scalar_mul(
                            qP[h][:, 1, tcols], res[:], PK_SCALE)

                ps = ps1.tile([128, TT], f32, tag="ps")
                proj(wkv16, wkv8, 0, 128, ps)
                nc.scalar.copy(khb[0:64, tcols], ps[0:64, :])
                res = sp1.tile([64, TT], f32, tag="qres")
                nc.vector.scalar_tensor_tensor(
                    res[:], in0=ps[0:64, :], scalar=1.0,
                    in1=khb[0:64, tcols], op0=Alu.mult, op1=Alu.subtract)
                nc.gpsimd.tensor_scalar_mul(
                    kP[:, 0, tcols], res[:], PK_SCALE)
                nc.gpsimd.tensor_scalar_mul(
                    kP[:, 1, tcols], khb[0:64, tcols], 1.0 / PK_SCALE)
                vtmp = sp1.tile([64, TT], f32, tag="vtmp")
                nc.scalar.copy(vtmp[:], ps[64:128, :])
                for j in range(sub):
                    ptr = psst.tile([128, TT], f32, tag="st")
                    nc.tensor.transpose(ptr[0:128, 0:64],
                                        vtmp[:, j * 128:(j + 1) * 128],
                                        ident[0:64, 0:64])
                    ch = tt * sub + j
                    nc.scalar.copy(vhat[:, ch * 65:ch * 65 + 64],
                                   ptr[0:128, 0:64])
                # batch-0 S~ blocks for the q-chunks this tile enabled
                bb, ltt = tt // tt_per_b, tt % tt_per_b
                if bb == 0:
                    for h in range(HEADS_PER_CORE):
                        for qc in range(ltt * sub, (ltt + 1) * sub):
                            s_block(bb, h, qc)
                    if ltt == tt_per_b - 1:
                        for h in range(HEADS_PER_CORE):
                            s_final(bb, h)

            ph1.close()
            # ============ phase 2 + per-batch o_proj ====================
            load_wo8()
            aop = ph.enter_context(tc.tile_pool(name="aop", bufs=1))
            ptp = ph.enter_context(tc.tile_pool(name="ptp", bufs=4))
            outp = ph.enter_context(tc.tile_pool(name="outp", bufs=2))
            # ao[c]: fp8 pack [128, 2(level), tokens]
            ao = [aop.tile([128, 2, tokens], f8, tag=f"ao{i}", name=f"ao{i}")
                  for i in range(n_oc)]

            def av(pav, pt, lo, w, bb, kc, nchunks):
                ch = bb * (s // 128) + kc
                nc.tensor.matmul(
                    pav[:, lo:lo + w], lhsT=vhat[:, ch * 65:ch * 65 + 65],
                    rhs=pt[:, lo:lo + w],
                    start=(kc == 0), stop=(kc == nchunks - 1),
                    skip_group_check=True)

            b1q = []
            for h in range(HEADS_PER_CORE):
                for qc in range(n_qc):
                    b1q.append(lambda h=h, qc=qc: s_block(1, h, qc))
                b1q.append(lambda h=h: s_final(1, h))
            # weight S~ pacing by main-slot size (qt+1 chunks of work)
            wsum = HEADS_PER_CORE * n_qt * (n_qt + 1) // 2
            bcum, acc = [], 0.0
            for h in range(HEADS_PER_CORE):
                for qt in range(n_qt):
                    acc += (qt + 1) * len(b1q) / wsum
                    bcum.append(min(int(round(acc)), len(b1q)))
            bcum[-1] = len(b1q)

            def oproj_part(bb, m, qt0, nqt):
                boff = bb * s
                osb = outp.tile([128, nqt * TT], f16, tag="ot", name="osb")
                for i in range(nqt):
                    for half in range(2):
                        po = ps1.tile([128, TT], f32, tag="ps")
                        qsl = slice(boff + (qt0 + i) * TT + half * 256,
                                    boff + (qt0 + i) * TT + half * 256 + 256)
                        for ci in range(n_oc):
                            nc.tensor.matmul(
                                po[:, 0:256],
                                lhsT=wo8[ci][:, :, m * 128:m * 128 + 128],
                                rhs=ao[ci][:, :, qsl],
                                start=(ci == 0), stop=(ci == n_oc - 1),
                                perf_mode=DR)
                        dst = osb[:, i * TT + half * 256:
                                  i * TT + half * 256 + 256]
                        if (m + i + half) % 2 == 0:
                            nc.vector.tensor_copy(dst, po[:, 0:256])
                        else:
                            nc.scalar.copy(dst, po[:, 0:256])
                nc.sync.dma_start(
                    out=out_d[m * 128:(m + 1) * 128,
                              boff + qt0 * TT:boff + (qt0 + nqt) * TT],
                    in_=osb[:])

            out_r = out_d.rearrange("(mm p) t -> p mm t", p=128)

            def oproj_b1_group(qt, g):
                """4 consecutive m-blocks of batch-1 q-tile qt, one DMA."""
                boff = b * s - s
                osb = outp.tile([128, 4, TT], f16, tag="og", name="osbg")
                for mi in range(4):
                    m = g * 4 + mi
                    for half in range(2):
                        po = ps1.tile([128, TT], f32, tag="ps")
                        qsl = slice(boff + qt * TT + half * 256,
                                    boff + qt * TT + half * 256 + 256)
                        for ci in range(n_oc):
                            nc.tensor.matmul(
                                po[:, 0:256],
                                lhsT=wo8[ci][:, :, m * 128:m * 128 + 128],
                                rhs=ao[ci][:, :, qsl],
                                start=(ci == 0), stop=(ci == n_oc - 1),
                                perf_mode=DR)
                        dst = osb[:, mi, half * 256:half * 256 + 256]
                        if (m + half) % 2 == 0:
                            nc.vector.tensor_copy(dst, po[:, 0:256])
                        else:
                            nc.scalar.copy(dst, po[:, 0:256])
                nc.sync.dma_start(
                    out=out_r[:, g * 4:g * 4 + 4,
                              boff + qt * TT:boff + (qt + 1) * TT],
                    in_=osb[:])

            def attn_slot(bb, h, qt):
                boff = bb * s
                qlo = boff + qt * TT
                pav = psav.tile([65, TT], f32, tag="pav")
                nchunks = (qt + 1) * sub
                prev = None
                for kc in range(nchunks):
                    ksl = slice(boff + kc * 128, boff + kc * 128 + 128)
                    j = kc - qt * sub
                    lo = max(j, 0) * 128  # cols < lo fully masked
                    w = TT - lo
                    s2 = ps1.tile([128, TT], f32, tag="ps")
                    nc.tensor.matmul(
                        s2[:, lo:lo + w], lhsT=khb[:, ksl],
                        rhs=qA[h][:, qlo + lo:qlo + TT],
                        start=True, stop=False)
                    # fp8 DoubleRow lo-pass (<=256-wide halves)
                    nhalf = (w + 255) // 256
                    for hf in range(nhalf):
                        hlo = lo + hf * 256
                        hw = min(256, TT - hlo)
                        nc.tensor.matmul(
                            s2[:, hlo:hlo + hw],
                            lhsT=kP[:, :, ksl],
                            rhs=qP[h][:, :, qlo + hlo:qlo + hlo + hw],
                            start=False,
                            stop=(j < 0 and hf == nhalf - 1),
                            perf_mode=DR, skip_group_check=True)
                    if j >= 0:
                        nc.tensor.matmul(
                            s2[:, lo:lo + 128], lhsT=identM[:],
                            rhs=maskKQ[:], start=False, stop=True,
                            skip_group_check=True)
                    pt = ptp.tile([128, TT], f32r, tag="pt")
                    nc.scalar.activation(pt[:, lo:lo + w],
                                         s2[:, lo:lo + w], Act.Exp)
                    if prev is not None:
                        av(pav, prev[0], prev[1], prev[2], bb,
                           prev[3], nchunks)
                    prev = (pt, lo, w, kc)
                av(pav, prev[0], prev[1], prev[2], bb, prev[3], nchunks)

                # pav row 64 = l/4; rec = 4/l; bcs = 0.0625*4/l
                # = 0.25/l, so aof = PV * bcs = attn/4 (fp8-safe)
                rec = mp.tile([65, TT], f32r, tag="rec")
                with nc.allow_low_precision(
                        reason="1/l broadcast feeds fp32r matmul"):
                    nc.vector.reciprocal(rec[64:65, :], pav[64:65, :])
                bc = psbc.tile([64, TT], f32, tag="bc")
                nc.tensor.matmul(
                    bc[:], lhsT=onesc[64:65, 0:HD],
                    rhs=rec[64:65, :], start=True, stop=True)
                bcs = mp.tile([64, TT], f32, tag="bcs")
                nc.scalar.copy(bcs[:], bc[:])
                # AO0 = fp8(aof), AO1 = fp8(aof - AO0); wo8 = 4*tern
                # aof half matches ao's base partition (SB+SB ops
                # require equal base partitions)
                rows = slice((h % 2) * 64, (h % 2) * 64 + 64)
                aof = mp.tile([128, TT], f32, tag="aof")
                nc.vector.tensor_tensor(
                    aof[rows, :], pav[0:64, :], bcs[:], op=Alu.mult)
                c = h // 2
                nc.gpsimd.tensor_copy(
                    ao[c][rows, 0, qlo:qlo + TT], aof[rows, :])
                nc.gpsimd.scalar_tensor_tensor(
                    ao[c][rows, 1, qlo:qlo + TT],
                    in0=ao[c][rows, 0, qlo:qlo + TT], scalar=-1.0,
                    in1=aof[rows, :], op0=Alu.mult, op1=Alu.add)

            # batch 0: h-major, batch-1 S~ ridesharing in slot gaps
            slot = 0
            for h in range(HEADS_PER_CORE):
                for qt in range(n_qt):
                    attn_slot(0, h, qt)
                    lo_i = bcum[slot - 1] if slot else 0
                    for fn in b1q[lo_i:bcum[slot]]:
                        fn()
                    slot += 1
            # batch 1: qt-major so its o_proj can interleave per q-tile;
            # batch-0 o_proj spread over the slots
            per = (n_mt + HEADS_PER_CORE * n_qt - 1) \
                // (HEADS_PER_CORE * n_qt)
            for qt in range(n_qt):
                for h in range(HEADS_PER_CORE):
                    attn_slot(1, h, qt)
                    si = qt * HEADS_PER_CORE + h
                    for m in range(si * per, min((si + 1) * per, n_mt)):
                        oproj_part(0, m, 0, n_qt)
                    if qt > 0:
                        # previous q-tile's batch-1 o_proj rides this slot
                        oproj_b1_group(qt - 1, h)
            for g in range(HEADS_PER_CORE):
                oproj_b1_group(n_qt - 1, g)

    nc.compile()
    return nc


def _ternarize(w):
    w = np.asarray(w, np.float32)
    scale = max(np.abs(w).mean(), 1e-6)
    return ((w > 0.05 * scale).astype(np.float32)
            - (w < -0.05 * scale).astype(np.float32))


def kernel(x, wq, wk, wv, wo):
    from concourse.bass_utils import run_bass_kernel_spmd

    if "nc" not in _CACHE:
        _CACHE["nc"] = _build_program()
    nc = _CACHE["nc"]

    tq = _ternarize(wq)
    tk = _ternarize(wk)
    tv = _ternarize(wv)
    to = _ternarize(wo)

    xT = np.ascontiguousarray(np.asarray(x, np.float32).reshape(B * S, D).T)
    xh = xT.astype(np.float16)
    xl = ((xT - xh.astype(np.float32)) * XL_SCALE).astype(E4)

    in_maps = []
    for c in range(NCORES):
        qsl = slice(c * QROWS, (c + 1) * QROWS)
        ksl = slice(c * HD, (c + 1) * HD)
        wkv = np.concatenate([tk[ksl], tv[ksl]], axis=0)  # [128, D]
        wqT = np.ascontiguousarray(tq[qsl].T)
        wkvT = np.ascontiguousarray(wkv.T)
        in_maps.append({
            "xh": xh, "xl": xl,
            "wq_hi": wqT.astype(np.float16),
            "wq_lo": (wqT / XL_SCALE).astype(E4),
            "wkv_hi": wkvT.astype(np.float16),
            "wkv_lo": (wkvT / XL_SCALE).astype(E4),
            "wo8": np.ascontiguousarray(to[:, qsl].T * 4.0).astype(E4),
        })

    res = run_bass_kernel_spmd(nc, in_maps, list(range(NCORES)))
    total = res.results[0]["out"].astype(np.float32)
    for c in range(1, NCORES):
        total = total + res.results[c]["out"].astype(np.float32)
    return np.ascontiguousarray(total.T).reshape(B, S, D).astype(np.float32)


# revision 42
# speedup vs baseline: 1.3880x; 1.0913x over previous
"""GQA causal attention (ternary weights) on 8 TRN2 NeuronCores.

Strategy (tensor-parallel over heads, per sharding hint):
  - core c owns Q heads [4c, 4c+4) and KV head c.
  - host: ternarize weights; split x into fp16 hi + fp8(e4m3, x512) residual;
    fp8 weight copies are ternary/512 (exact: 2^-9 is an e4m3 subnormal) so
    the fp8 DoubleRow lo-pass lands at natural scale in the same PSUM group
    as the fp16 hi-pass.
  - device per core:
      phase 1: q/k/v projections: fp16 hi matmuls + fp8 DoubleRow lo matmuls
               (2 contraction chunks per instruction, half cost). Activations
               are stored as fp16 (qA rows 0:64 = q/8, khb rows 0:64 = k,
               row 64 = bias) plus fp8 packs for the attention lo-pass:
               q-pack = (qh/64, qres*64), k-pack = (kres*64, kh/64).
      phase 2: per (batch, head): fp16 S~=QK^T in [q,k] layout for row-max
               (diagonal causal mask folded in as an identity x mask-const
               matmul on the PE); exact S^T in [k,q] via fp16 hi matmul
               (with folded -max bias row) + one fp8 DoubleRow lo matmul;
               exp on ScalarE; PV + row-sums via fp32r matmul with a 0.25
               column appended to V (so the normalizer is 1/(4l)).
      phase 3: o_proj via 2-level fp8 DoubleRow: AO0=fp8(ao/4),
               AO1=fp8(ao/4-AO0), weights 4*ternary (exact in fp8); fp16
               partial outputs DMA'd per batch so they overlap attention.
  - host: sum the 8 fp16 partial outputs in fp32 (row-split "all-reduce").
"""

import sys

sys.path.insert(0, "/opt/trn_rl_repo")

import numpy as np
import ml_dtypes

B = 2
S = 2048
D = 2048
NCORES = 8
HEADS_PER_CORE = 4
HD = 64
QROWS = HEADS_PER_CORE * HD  # 256
TT = 512  # token tile
MASK_NEG = -30000.0
XL_SCALE = 512.0     # x residual fp8 scale (weights get 1/512)
PK_SCALE = 64.0      # S lo-pass pack scale (carrier /64, residual x64)

E4 = ml_dtypes.float8_e4m3

_CACHE = {}


def _build_program(b=B, s=S, d=D):
    import concourse.bacc as bacc
    import concourse.tile as tile
    import concourse.mybir as mybir
    from concourse import masks
    from contextlib import ExitStack

    f32 = mybir.dt.float32
    f32r = mybir.dt.float32r
    f16 = mybir.dt.float16
    f8 = mybir.dt.float8e4
    Alu = mybir.AluOpType
    Act = mybir.ActivationFunctionType
    DR = mybir.MatmulPerfMode.DoubleRow

    tokens = b * s
    n_tt = tokens // TT          # token tiles
    tt_per_b = s // TT
    n_dc = d // 128              # contraction chunks for projections
    n_qt = s // TT               # 512-wide q tiles per batch
    n_qc = s // 128              # 128-wide q chunks per batch (max pass)
    n_mt = d // 128              # output row tiles for o_proj
    n_oc = QROWS // 128          # o_proj contraction chunks (2)
    sub = TT // 128              # 128-sub-blocks per 512 tile (4)

    nc = bacc.Bacc("TRN2", target_bir_lowering=False, debug=False,
                   num_devices=NCORES)

    xh_d = nc.dram_tensor("xh", [d, tokens], f16, kind="ExternalInput").ap()
    xl_d = nc.dram_tensor("xl", [d, tokens], f8, kind="ExternalInput").ap()
    wqh_d = nc.dram_tensor("wq_hi", [d, QROWS], f16, kind="ExternalInput").ap()
    wql_d = nc.dram_tensor("wq_lo", [d, QROWS], f8, kind="ExternalInput").ap()
    wkh_d = nc.dram_tensor("wkv_hi", [d, 128], f16, kind="ExternalInput").ap()
    wkl_d = nc.dram_tensor("wkv_lo", [d, 128], f8, kind="ExternalInput").ap()
    wo_d = nc.dram_tensor("wo8", [QROWS, d], f8, kind="ExternalInput").ap()
    out_d = nc.dram_tensor("out", [d, tokens], f16, kind="ExternalOutput").ap()

    with tile.TileContext(nc) as tc, ExitStack() as top:
        constp = top.enter_context(tc.tile_pool(name="const", bufs=1))
        wpool = top.enter_context(tc.tile_pool(name="wts", bufs=1))
        pp = top.enter_context(tc.tile_pool(name="persist", bufs=1))

        # --- constants -------------------------------------------------
        # maskKQ[p,q] = MASK_NEG where p > q (keep k<=q in [k,q] layout)
        maskKQ = constp.tile([128, 128], f16, tag="maskKQ")
        nc.gpsimd.memset(maskKQ[:], 0.0)
        nc.gpsimd.affine_select(
            out=maskKQ[:], in_=maskKQ[:], compare_op=Alu.is_ge, fill=MASK_NEG,
            base=0, pattern=[[1, 128]], channel_multiplier=-1)
        # maskQK[q,k] = MASK_NEG where k > q ([q,k] layout)
        maskQK = constp.tile([128, 128], f16, tag="maskQK")
        nc.gpsimd.memset(maskQK[:], 0.0)
        nc.gpsimd.affine_select(
            out=maskQK[:], in_=maskQK[:], compare_op=Alu.is_ge, fill=MASK_NEG,
            base=0, pattern=[[-1, 128]], channel_multiplier=1)
        identM = constp.tile([128, 128], f16, tag="identM")
        masks.make_identity(nc, identM[:])
        ident = constp.tile([128, 128], f32, tag="ident")
        masks.make_identity(nc, ident[:])


        # --- weights (DMA order tuned so tile-0 compute starts early) ---
        wq16 = wpool.tile([128, n_dc * QROWS], f16, tag="wq16", name="wq16")
        nc.sync.dma_start(
            out=wq16[:].rearrange("p (c n) -> p c n", n=QROWS)[:, :, 0:128],
            in_=wqh_d.rearrange("(c p) n -> p c n", p=128)[:, :, 0:128])
        wq8 = wpool.tile([128, n_dc, QROWS], f8, tag="wq8", name="wq8")
        wkv16 = wpool.tile([128, n_dc * 128], f16, tag="wkv16", name="wkv16")
        wkv8 = wpool.tile([128, n_dc, 128], f8, tag="wkv8", name="wkv8")

        def load_weights_rest():
            nc.sync.dma_start(
                out=wq16[:].rearrange(
                    "p (c n) -> p c n", n=QROWS)[:, :, 128:QROWS],
                in_=wqh_d.rearrange("(c p) n -> p c n", p=128)[:, :,
                                                              128:QROWS])
            nc.sync.dma_start(
                out=wq8[:], in_=wql_d.rearrange("(c p) n -> p c n", p=128))
            nc.sync.dma_start(
                out=wkv16[:].rearrange("p (c n) -> p c n", n=128),
                in_=wkh_d.rearrange("(c p) n -> p c n", p=128))
            nc.sync.dma_start(
                out=wkv8[:], in_=wkl_d.rearrange("(c p) n -> p c n", p=128))

        # o_proj weights: [128, 2(level), d] per contraction chunk; both
        # levels are the same 4*ternary data. Loaded at end of phase 1.
        wo8 = [wpool.tile([128, 2, d], f8, tag=f"wo8_{c}", name=f"wo8_{c}")
               for c in range(n_oc)]

        def load_wo8():
            for c in range(n_oc):
                for lv in range(2):
                    nc.sync.dma_start(
                        out=wo8[c][:, lv, :],
                        in_=wo_d[c * 128:(c + 1) * 128, :])

        # --- persistent activations -----------------------------------
        # qA[h]: rows 0:64 = fp16(q/8), row 64 = m~ bias (max pass)
        # qP[h]: fp8 pack [64, 2, tokens] = (qh/64, qres*64)
        # khb:   rows 0:64 = fp16(k), row 64 = -1
        # kP:    fp8 pack [64, 2, tokens] = (kres*64, kh/64)
        # vhat:  [128, chunk*65]: cols 0:64 of chunk = v, col 64 = 0.25
        qA = [pp.tile([65, tokens], f16, tag=f"qA{h}", name=f"qA{h}")
              for h in range(HEADS_PER_CORE)]
        qP = [pp.tile([64, 2, tokens], f8, tag=f"qP{h}", name=f"qP{h}")
              for h in range(HEADS_PER_CORE)]
        khb = pp.tile([65, tokens], f16, tag="khb")
        kP = pp.tile([64, 2, tokens], f8, tag="kP")
        n_ch = tokens // 128
        vhat = pp.tile([128, n_ch * 65], f32r, tag="vhat")
        nc.scalar.activation(
            vhat[:], ident[:, 0:1].to_broadcast([128, n_ch * 65]),
            Act.Copy, bias=0.25, scale=0.0)
        nc.gpsimd.memset(khb[64:65, :], -1.0)

        with ExitStack() as ph:
            mp = ph.enter_context(tc.tile_pool(name="mp", bufs=3))
            ps1 = ph.enter_context(
                tc.tile_pool(name="ps1", bufs=5, space="PSUM"))
            psst = ph.enter_context(
                tc.tile_pool(name="psst", bufs=2, space="PSUM"))
            psav = ph.enter_context(
                tc.tile_pool(name="psav", bufs=1, space="PSUM"))

            # ---------- S~ max-estimate pass, as schedulable blocks ------
            mstate = {}
            mbp = ph.enter_context(tc.tile_pool(name="mbp", bufs=8))

            def s_block(bb, h, qc):
                boff = bb * s
                if qc == 0:
                    mstate[(bb, h)] = mbp.tile([128, n_qc], f32, tag="mbuf",
                                               name="mbuf")
                mbuf = mstate[(bb, h)]
                qsl = slice(boff + qc * 128, boff + qc * 128 + 128)
                ntk = qc // sub + 1
                mtmp = mp.tile([128, 8], f32, tag="mtmp")
                for kt in range(ntk):
                    w = min(TT, (qc + 1) * 128 - kt * TT)
                    st = psst.tile([128, TT], f32, tag="st")
                    nc.tensor.matmul(
                        st[:, 0:w],
                        lhsT=qA[h][0:64, qsl],
                        rhs=khb[0:64, boff + kt * TT:boff + kt * TT + w],
                        start=True, stop=(kt != ntk - 1))
                    if kt == ntk - 1:  # diagonal block is last 128 cols
                        nc.tensor.matmul(
                            st[:, w - 128:w], lhsT=identM[:], rhs=maskQK[:],
                            start=False, stop=True, skip_group_check=True)
                    nc.vector.tensor_reduce(
                        mtmp[:, kt:kt + 1], st[:, 0:w],
                        axis=mybir.AxisListType.X, op=Alu.max)
                nc.vector.tensor_reduce(
                    mbuf[:, qc:qc + 1], mtmp[:, 0:ntk],
                    axis=mybir.AxisListType.X, op=Alu.max)

            def s_final(bb, h):
                boff = bb * s
                mbuf = mstate.pop((bb, h))
                mps = psst.tile([128, TT], f32, tag="st")
                nc.tensor.transpose(mps[0:n_qc, 0:128], mbuf[:, 0:n_qc],
                                    ident[:, 0:128])
                mrow = mp.tile([n_qc, 128], f32, tag="mrow")
                nc.vector.tensor_copy(mrow[:], mps[0:n_qc, 0:128])
                nc.gpsimd.dma_start(
                    out=qA[h][64:65, boff:boff + s].rearrange(
                        "o (c t) -> o c t", t=128),
                    in_=mrow[:])

            # ================= phase 1: projections ====================
            ph1 = ExitStack()
            xp = ph1.enter_context(tc.tile_pool(name="xp", bufs=2))
            x8p = ph1.enter_context(tc.tile_pool(name="x8p", bufs=2))
            sp1 = ph1.enter_context(tc.tile_pool(name="sp1", bufs=3))
            for tt in range(n_tt):
                tcols = slice(tt * TT, (tt + 1) * TT)
                xt = xp.tile([128, n_dc * TT], f16, tag="x", name="xtile")
                xhr = xh_d.rearrange("(c p) t -> p c t", p=128)
                for ch in range(2):
                    cs = slice(ch * n_dc // 2, (ch + 1) * n_dc // 2)
                    nc.sync.dma_start(
                        out=xt[:].rearrange("p (c t) -> p c t", t=TT)[:, cs],
                        in_=xhr[:, cs, tcols])
                x8 = x8p.tile([128, n_dc, TT], f8, tag="x8", name="x8tile")
                xlr = xl_d.rearrange("(c p) t -> p c t", p=128)
                for ch in range(2):
                    cs = slice(ch * n_dc // 2, (ch + 1) * n_dc // 2)
                    nc.sync.dma_start(
                        out=x8[:, cs], in_=xlr[:, cs, tcols])
                if tt == 0:
                    load_weights_rest()

                def proj(w16, w8, mcol, mwid, ps):
                    for c in range(n_dc):
                        nc.tensor.matmul(
                            ps[:],
                            lhsT=w16[:, c * mwid + mcol:c * mwid + mcol + 128],
                            rhs=xt[:, c * TT:(c + 1) * TT],
                            start=(c == 0), stop=False)
                    for half in range(2):
                        hs = slice(half * 256, half * 256 + 256)
                        for cp in range(n_dc // 2):
                            nc.tensor.matmul(
                                ps[:, hs],
                                lhsT=w8[:, 2 * cp:2 * cp + 2,
                                        mcol:mcol + 128],
                                rhs=x8[:, 2 * cp:2 * cp + 2, hs],
                                start=False,
                                stop=(half == 1 and cp == n_dc // 2 - 1),
                                perf_mode=DR, skip_group_check=True)

                for m in range(QROWS // 128):
                    ps = ps1.tile([128, TT], f32, tag="ps")
                    proj(wq16, wq8, m * 128, QROWS, ps)
                    for i in range(2):
                        h = 2 * m + i
                        rows = slice(i * 64, i * 64 + 64)
                        # qA = fp16(q/8)
                        nc.scalar.activation(qA[h][0:64, tcols], ps[rows, :],
                                             Act.Copy, scale=0.125)
                        # res = q/8 - qA (fp16: keeps gpsimd inputs 16-bit)
                        res = sp1.tile([64, TT], f16, tag="qres")
                        nc.vector.scalar_tensor_tensor(
                            res[:], in0=ps[rows, :], scalar=0.125,
                            in1=qA[h][0:64, tcols],
                            op0=Alu.mult, op1=Alu.subtract)
                        # fp8 pack (SBUF->SBUF on gpsimd)
                        nc.gpsimd.tensor_scalar_mul(
                            qP[h][:, 0, tcols], qA[h][0:64, tcols],
                            1.0 / PK_SCALE)
                        nc.gpsimd.tensor_scalar_mul(
                            qP[h][:, 1, tcols], res[:], PK_SCALE)

                ps = ps1.tile([128, TT], f32, tag="ps")
                proj(wkv16, wkv8, 0, 128, ps)
                nc.scalar.copy(khb[0:64, tcols], ps[0:64, :])
                res = sp1.tile([64, TT], f16, tag="qres")
                nc.vector.scalar_tensor_tensor(
                    res[:], in0=ps[0:64, :], scalar=1.0,
                    in1=khb[0:64, tcols], op0=Alu.mult, op1=Alu.subtract)
                nc.gpsimd.tensor_scalar_mul(
                    kP[:, 0, tcols], res[:], PK_SCALE)
                nc.gpsimd.tensor_scalar_mul(
                    kP[:, 1, tcols], khb[0:64, tcols], 1.0 / PK_SCALE)
                vtmp = sp1.tile([64, TT], f32, tag="vtmp")
                nc.scalar.copy(vtmp[:], ps[64:128, :])
                for j in range(sub):
                    ptr = psst.tile([128, TT], f32, tag="st")
                    nc.tensor.transpose(ptr[0:128, 0:64],
                                        vtmp[:, j * 128:(j + 1) * 128],
                                        ident[0:64, 0:64])
                    ch = tt * sub + j
                    nc.scalar.copy(vhat[:, ch * 65:ch * 65 + 64],
                                   ptr[0:128, 0:64])
                # batch-0 S~ blocks for the q-chunks this tile enabled
                bb, ltt = tt // tt_per_b, tt % tt_per_b
                if bb == 0:
                    for h in range(HEADS_PER_CORE):
                        for qc in range(ltt * sub, (ltt + 1) * sub):
                            s_block(bb, h, qc)
                    if ltt == tt_per_b - 1:
                        for h in range(HEADS_PER_CORE):
                            s_final(bb, h)

            ph1.close()
            # ============ phase 2 + per-batch o_proj ====================
            load_wo8()
            aop = ph.enter_context(tc.tile_pool(name="aop", bufs=1))
            ptp = ph.enter_context(tc.tile_pool(name="ptp", bufs=6))
            outp = ph.enter_context(tc.tile_pool(name="outp", bufs=3))
            # ao[c]: fp8 pack [128, 2(level), tokens]
            ao = [aop.tile([128, 2, tokens], f8, tag=f"ao{i}", name=f"ao{i}")
                  for i in range(n_oc)]

            def av(pav, pt, lo, w, bb, kc, nchunks):
                ch = bb * (s // 128) + kc
                nc.tensor.matmul(
                    pav[:, lo:lo + w], lhsT=vhat[:, ch * 65:ch * 65 + 65],
                    rhs=pt[:, lo:lo + w],
                    start=(kc == 0), stop=(kc == nchunks - 1),
                    skip_group_check=True)

            b1q = []
            for h in range(HEADS_PER_CORE):
                for qc in range(n_qc):
                    b1q.append(lambda h=h, qc=qc: s_block(1, h, qc))
                b1q.append(lambda h=h: s_final(1, h))
            # weight S~ pacing by main-slot size (qt+1 chunks of work);
            # slots run qt-major
            wsum = HEADS_PER_CORE * n_qt * (n_qt + 1) // 2
            bcum, acc = [], 0.0
            for qt in range(n_qt):
                for h in range(HEADS_PER_CORE):
                    acc += (qt + 1) * len(b1q) / wsum
                    bcum.append(min(int(round(acc)), len(b1q)))
            bcum[-1] = len(b1q)

            out_r = out_d.rearrange("(mm p) t -> p mm t", p=128)

            def oproj_group(bb, qt, g, eng):
                """4 consecutive m-blocks of one q-tile, one DMA out."""
                boff = bb * s
                osb = outp.tile([128, 4, TT], f16, tag="og", name="osbg")
                for mi in range(4):
                    m = g * 4 + mi
                    po = ps1.tile([128, TT], f32, tag="ps")
                    for half in range(2):
                        hs = slice(half * 256, half * 256 + 256)
                        qsl = slice(boff + qt * TT + half * 256,
                                    boff + qt * TT + half * 256 + 256)
                        for ci in range(n_oc):
                            nc.tensor.matmul(
                                po[:, hs],
                                lhsT=wo8[ci][:, :, m * 128:m * 128 + 128],
                                rhs=ao[ci][:, :, qsl],
                                start=(ci == 0), stop=(ci == n_oc - 1),
                                perf_mode=DR,
                                skip_group_check=(half == 1))
                    dst = osb[:, mi, :]
                    if eng == "v":
                        nc.vector.tensor_copy(dst, po[:])
                    else:
                        nc.scalar.copy(dst, po[:])
                nc.sync.dma_start(
                    out=out_r[:, g * 4:g * 4 + 4,
                              boff + qt * TT:boff + (qt + 1) * TT],
                    in_=osb[:])

            def attn_slot(bb, h, qt):
                boff = bb * s
                qlo = boff + qt * TT
                pav = psav.tile([65, TT], f32, tag="pav")
                nchunks = (qt + 1) * sub
                prev = None
                for kc in range(nchunks):
                    ksl = slice(boff + kc * 128, boff + kc * 128 + 128)
                    j = kc - qt * sub
                    lo = max(j, 0) * 128  # cols < lo fully masked
                    w = TT - lo
                    s2 = ps1.tile([128, TT], f32, tag="ps")
                    nc.tensor.matmul(
                        s2[:, lo:lo + w], lhsT=khb[:, ksl],
                        rhs=qA[h][:, qlo + lo:qlo + TT],
                        start=True, stop=False)
                    # fp8 DoubleRow lo-pass (<=256-wide halves)
                    nhalf = (w + 255) // 256
                    for hf in range(nhalf):
                        hlo = lo + hf * 256
                        hw = min(256, TT - hlo)
                        nc.tensor.matmul(
                            s2[:, hlo:hlo + hw],
                            lhsT=kP[:, :, ksl],
                            rhs=qP[h][:, :, qlo + hlo:qlo + hlo + hw],
                            start=False,
                            stop=(j < 0 and hf == nhalf - 1),
                            perf_mode=DR, skip_group_check=True)
                    if j >= 0:
                        nc.tensor.matmul(
                            s2[:, lo:lo + 128], lhsT=identM[:],
                            rhs=maskKQ[:], start=False, stop=True,
                            skip_group_check=True)
                    pt = ptp.tile([128, TT], f32r, tag="pt")
                    nc.scalar.activation(pt[:, lo:lo + w],
                                         s2[:, lo:lo + w], Act.Exp)
                    if prev is not None:
                        av(pav, prev[0], prev[1], prev[2], bb,
                           prev[3], nchunks)
                    prev = (pt, lo, w, kc)
                av(pav, prev[0], prev[1], prev[2], bb, prev[3], nchunks)

                # pav row 64 = l/4; rec = 4/l; broadcast on gpsimd; then
                # aof = PV * 4/l = 4*attn (fp16). AO0 = fp8(aof/16)
                # = fp8(attn/4), AO1 = fp8(aof/16 - AO0) = attn/4 - AO0.
                # Both wo8 levels are 4*ternary.
                # Pool only sees fp16->fp8 ops; the mixed stt runs on DVE.
                rec = mp.tile([1, TT], f32, tag="rec")
                with nc.allow_low_precision(
                        reason="1/l broadcast feeds fp8 conversions"):
                    nc.vector.reciprocal(rec[:], pav[64:65, :])
                bcs = mp.tile([64, TT], f32, tag="bcs")
                nc.gpsimd.partition_broadcast(bcs[:], rec[:])
                # aof half matches ao's base partition (SB+SB ops
                # require equal base partitions)
                rows = slice((h % 2) * 64, (h % 2) * 64 + 64)
                aof = mp.tile([128, TT], f16, tag="aof")
                nc.vector.tensor_tensor(
                    aof[rows, :], pav[0:64, :], bcs[:], op=Alu.mult)
                c = h // 2
                nc.gpsimd.tensor_scalar_mul(
                    ao[c][rows, 0, qlo:qlo + TT], aof[rows, :], 1.0 / 16)
                nc.vector.scalar_tensor_tensor(
                    ao[c][rows, 1, qlo:qlo + TT],
                    in0=aof[rows, :], scalar=1.0 / 16,
                    in1=ao[c][rows, 0, qlo:qlo + TT],
                    op0=Alu.mult, op1=Alu.subtract)

            # Both batches qt-major: each q-tile's o_proj rides the next
            # q-tile's attention slots. Batch-0 slots also carry the
            # batch-1 S~ rideshare; batch-0's o_proj copies go to ScalarE
            # (DVE is reduce-bound there), batch-1's to DVE (ScalarE is
            # exp-bound there).
            slot = 0
            for qt in range(n_qt):
                for h in range(HEADS_PER_CORE):
                    attn_slot(0, h, qt)
                    lo_i = bcum[slot - 1] if slot else 0
                    for fn in b1q[lo_i:bcum[slot]]:
                        fn()
                    slot += 1
                    if qt > 0:
                        oproj_group(0, qt - 1, h, "s")
            for qt in range(n_qt):
                for h in range(HEADS_PER_CORE):
                    attn_slot(1, h, qt)
                    if qt == 0:
                        oproj_group(0, n_qt - 1, h, "s")
                    else:
                        oproj_group(1, qt - 1, h, "v")
            for g in range(HEADS_PER_CORE):
                oproj_group(1, n_qt - 1, g, "v")

    nc.compile()
    return nc


def _ternarize(w):
    w = np.asarray(w, np.float32)
    scale = max(np.abs(w).mean(), 1e-6)
    return ((w > 0.05 * scale).astype(np.float32)
            - (w < -0.05 * scale).astype(np.float32))


def kernel(x, wq, wk, wv, wo):
    from concourse.bass_utils import run_bass_kernel_spmd

    if "nc" not in _CACHE:
        _CACHE["nc"] = _build_program()
    nc = _CACHE["nc"]

    tq = _ternarize(wq)
    tk = _ternarize(wk)
    tv = _ternarize(wv)
    to = _ternarize(wo)

    xT = np.ascontiguousarray(np.asarray(x, np.float32).reshape(B * S, D).T)
    xh = xT.astype(np.float16)
    xl = ((xT - xh.astype(np.float32)) * XL_SCALE).astype(E4)

    in_maps = []
    for c in range(NCORES):
        qsl = slice(c * QROWS, (c + 1) * QROWS)
        ksl = slice(c * HD, (c + 1) * HD)
        wkv = np.concatenate([tk[ksl], tv[ksl]], axis=0)  # [128, D]
        wqT = np.ascontiguousarray(tq[qsl].T)
        wkvT = np.ascontiguousarray(wkv.T)
        in_maps.append({
            "xh": xh, "xl": xl,
            "wq_hi": wqT.astype(np.float16),
            "wq_lo": (wqT / XL_SCALE).astype(E4),
            "wkv_hi": wkvT.astype(np.float16),
            "wkv_lo": (wkvT / XL_SCALE).astype(E4),
            "wo8": np.ascontiguousarray(to[:, qsl].T * 4.0).astype(E4),
        })

    res = run_bass_kernel_spmd(nc, in_maps, list(range(NCORES)))
    total = res.results[0]["out"].astype(np.float32)
    for c in range(1, NCORES):
        total = total + res.results[c]["out"].astype(np.float32)
    return np.ascontiguousarray(total.T).reshape(B, S, D).astype(np.float32)


# revision 53
# speedup vs baseline: 1.4517x; 1.0459x over previous
"""GQA causal attention (ternary weights) on 8 TRN2 NeuronCores.

Strategy (tensor-parallel over heads, per sharding hint):
  - core c owns Q heads [4c, 4c+4) and KV head c.
  - host: ternarize weights; split x into fp16 hi + fp8(e4m3, x512) residual;
    fp8 weight copies are ternary/512 (exact: 2^-9 is an e4m3 subnormal) so
    the fp8 DoubleRow lo-pass lands at natural scale in the same PSUM group
    as the fp16 hi-pass.
  - device per core:
      phase 1: q/k/v projections: fp16 hi matmuls + fp8 DoubleRow lo matmuls
               (2 contraction chunks per instruction, half cost). Activations
               are stored as fp16 (qA rows 0:64 = q/8, khb rows 0:64 = k,
               row 64 = bias) plus fp8 packs for the attention lo-pass:
               q-pack = (qh/64, qres*64), k-pack = (kres*64, kh/64).
      phase 2: per (batch, head): fp16 S~=QK^T in [q,k] layout for row-max
               (diagonal causal mask folded in as an identity x mask-const
               matmul on the PE); exact S^T in [k,q] via fp16 hi matmul
               (with folded -max bias row) + one fp8 DoubleRow lo matmul;
               exp on ScalarE; PV + row-sums via fp32r matmul with a 0.25
               column appended to V (so the normalizer is 1/(4l)).
      phase 3: o_proj via 2-level fp8 DoubleRow: AO0=fp8(ao/4),
               AO1=fp8(ao/4-AO0), weights 4*ternary (exact in fp8); fp16
               partial outputs DMA'd per batch so they overlap attention.
  - host: sum the 8 fp16 partial outputs in fp32 (row-split "all-reduce").
"""

import sys

sys.path.insert(0, "/opt/trn_rl_repo")

import numpy as np
import ml_dtypes

B = 2
S = 2048
D = 2048
NCORES = 8
HEADS_PER_CORE = 4
HD = 64
QROWS = HEADS_PER_CORE * HD  # 256
TT = 512  # token tile
MASK_NEG = -30000.0
XL_SCALE = 512.0     # x residual fp8 scale (weights get 1/512)
PK_SCALE = 64.0      # S lo-pass pack scale (carrier /64, residual x64)

E4 = ml_dtypes.float8_e4m3

_CACHE = {}


def _build_program(b=B, s=S, d=D):
    import concourse.bacc as bacc
    import concourse.tile as tile
    import concourse.mybir as mybir
    from concourse import masks
    from contextlib import ExitStack

    f32 = mybir.dt.float32
    f32r = mybir.dt.float32r
    f16 = mybir.dt.float16
    f8 = mybir.dt.float8e4
    Alu = mybir.AluOpType
    Act = mybir.ActivationFunctionType
    DR = mybir.MatmulPerfMode.DoubleRow

    tokens = b * s
    n_tt = tokens // TT          # token tiles
    tt_per_b = s // TT
    n_dc = d // 128              # contraction chunks for projections
    n_qt = s // TT               # 512-wide q tiles per batch
    n_qc = s // 128              # 128-wide q chunks per batch (max pass)
    n_mt = d // 128              # output row tiles for o_proj
    n_oc = QROWS // 128          # o_proj contraction chunks (2)
    sub = TT // 128              # 128-sub-blocks per 512 tile (4)

    nc = bacc.Bacc("TRN2", target_bir_lowering=False, debug=False,
                   num_devices=NCORES)

    xh_d = nc.dram_tensor("xh", [d, tokens], f16, kind="ExternalInput").ap()
    xl_d = nc.dram_tensor("xl", [d, tokens], f8, kind="ExternalInput").ap()
    wqh_d = nc.dram_tensor("wq_hi", [d, QROWS], f16, kind="ExternalInput").ap()
    wql_d = nc.dram_tensor("wq_lo", [d, QROWS], f8, kind="ExternalInput").ap()
    wkh_d = nc.dram_tensor("wkv_hi", [d, 128], f16, kind="ExternalInput").ap()
    wkl_d = nc.dram_tensor("wkv_lo", [d, 128], f8, kind="ExternalInput").ap()
    wo_d = nc.dram_tensor("wo8", [QROWS, d], f8, kind="ExternalInput").ap()
    out_d = nc.dram_tensor("out", [d, tokens], f16, kind="ExternalOutput").ap()

    with tile.TileContext(nc) as tc, ExitStack() as top:
        constp = top.enter_context(tc.tile_pool(name="const", bufs=1))
        wpool = top.enter_context(tc.tile_pool(name="wts", bufs=1))
        pp = top.enter_context(tc.tile_pool(name="persist", bufs=1))

        # --- constants -------------------------------------------------
        # maskKQ[p,q] = MASK_NEG where p > q (keep k<=q in [k,q] layout)
        maskKQ = constp.tile([128, 128], f16, tag="maskKQ")
        nc.gpsimd.memset(maskKQ[:], 0.0)
        nc.gpsimd.affine_select(
            out=maskKQ[:], in_=maskKQ[:], compare_op=Alu.is_ge, fill=MASK_NEG,
            base=0, pattern=[[1, 128]], channel_multiplier=-1)
        # maskQK[q,k] = MASK_NEG where k > q ([q,k] layout)
        maskQK = constp.tile([128, 128], f16, tag="maskQK")
        nc.gpsimd.memset(maskQK[:], 0.0)
        nc.gpsimd.affine_select(
            out=maskQK[:], in_=maskQK[:], compare_op=Alu.is_ge, fill=MASK_NEG,
            base=0, pattern=[[-1, 128]], channel_multiplier=1)
        identM = constp.tile([128, 128], f16, tag="identM")
        masks.make_identity(nc, identM[:])
        ident = constp.tile([128, 128], f32, tag="ident")
        masks.make_identity(nc, ident[:])


        # --- weights (DMA order tuned so tile-0 compute starts early) ---
        wq16 = wpool.tile([128, n_dc * QROWS], f16, tag="wq16", name="wq16")
        nc.sync.dma_start(
            out=wq16[:].rearrange("p (c n) -> p c n", n=QROWS)[:, :, 0:128],
            in_=wqh_d.rearrange("(c p) n -> p c n", p=128)[:, :, 0:128])
        wq8 = wpool.tile([128, n_dc, QROWS], f8, tag="wq8", name="wq8")
        wkv16 = wpool.tile([128, n_dc * 128], f16, tag="wkv16", name="wkv16")
        wkv8 = wpool.tile([128, n_dc, 128], f8, tag="wkv8", name="wkv8")

        def load_weights_rest():
            nc.sync.dma_start(
                out=wq16[:].rearrange(
                    "p (c n) -> p c n", n=QROWS)[:, :, 128:QROWS],
                in_=wqh_d.rearrange("(c p) n -> p c n", p=128)[:, :,
                                                              128:QROWS])
            nc.sync.dma_start(
                out=wq8[:], in_=wql_d.rearrange("(c p) n -> p c n", p=128))
            nc.sync.dma_start(
                out=wkv16[:].rearrange("p (c n) -> p c n", n=128),
                in_=wkh_d.rearrange("(c p) n -> p c n", p=128))
            nc.sync.dma_start(
                out=wkv8[:], in_=wkl_d.rearrange("(c p) n -> p c n", p=128))

        # o_proj weights: [128, 2(level), d] per contraction chunk; both
        # levels are the same 4*ternary data. Loaded at end of phase 1.
        wo8 = [wpool.tile([128, 2, d], f8, tag=f"wo8_{c}", name=f"wo8_{c}")
               for c in range(n_oc)]

        def load_wo8():
            for c in range(n_oc):
                for lv in range(2):
                    nc.sync.dma_start(
                        out=wo8[c][:, lv, :],
                        in_=wo_d[c * 128:(c + 1) * 128, :])

        # --- persistent activations -----------------------------------
        # qA[h]: rows 0:64 = fp16(q/8), row 64 = m~ bias (max pass)
        # qP[h]: fp8 pack [64, 2, tokens] = (qh/64, qres*64)
        # khb:   rows 0:64 = fp16(k), row 64 = -1
        # kP:    fp8 pack [64, 2, tokens] = (kres*64, kh/64)
        # vhat:  [128, chunk*65]: cols 0:64 of chunk = v, col 64 = 0.25
        qA = [pp.tile([65, tokens], f16, tag=f"qA{h}", name=f"qA{h}")
              for h in range(HEADS_PER_CORE)]
        qP = [pp.tile([64, 2, tokens], f8, tag=f"qP{h}", name=f"qP{h}")
              for h in range(HEADS_PER_CORE)]
        khb = pp.tile([65, tokens], f16, tag="khb")
        kP = pp.tile([64, 2, tokens], f8, tag="kP")
        n_ch = tokens // 128
        vhat = pp.tile([128, n_ch * 65], f32r, tag="vhat")
        nc.scalar.activation(
            vhat[:], ident[:, 0:1].to_broadcast([128, n_ch * 65]),
            Act.Copy, bias=0.25, scale=0.0)
        nc.gpsimd.memset(khb[64:65, :], -1.0)

        with ExitStack() as ph:
            mp = ph.enter_context(tc.tile_pool(name="mp", bufs=3))
            ps1 = ph.enter_context(
                tc.tile_pool(name="ps1", bufs=5, space="PSUM"))
            psst = ph.enter_context(
                tc.tile_pool(name="psst", bufs=2, space="PSUM"))
            psav = ph.enter_context(
                tc.tile_pool(name="psav", bufs=1, space="PSUM"))

            # ---------- S~ max-estimate pass, as schedulable blocks ------
            mstate = {}
            mbp = ph.enter_context(tc.tile_pool(name="mbp", bufs=8))

            def s_block(bb, h, qc):
                boff = bb * s
                if qc == 0:
                    mstate[(bb, h)] = mbp.tile([128, n_qc], f32, tag="mbuf",
                                               name="mbuf")
                mbuf = mstate[(bb, h)]
                qsl = slice(boff + qc * 128, boff + qc * 128 + 128)
                ntk = qc // sub + 1
                mtmp = mp.tile([128, 8], f32, tag="mtmp")
                for kt in range(ntk):
                    w = min(TT, (qc + 1) * 128 - kt * TT)
                    st = psst.tile([128, TT], f32, tag="st")
                    nc.tensor.matmul(
                        st[:, 0:w],
                        lhsT=qA[h][0:64, qsl],
                        rhs=khb[0:64, boff + kt * TT:boff + kt * TT + w],
                        start=True, stop=(kt != ntk - 1))
                    if kt == ntk - 1:  # diagonal block is last 128 cols
                        nc.tensor.matmul(
                            st[:, w - 128:w], lhsT=identM[:], rhs=maskQK[:],
                            start=False, stop=True, skip_group_check=True)
                    nc.vector.tensor_reduce(
                        mtmp[:, kt:kt + 1], st[:, 0:w],
                        axis=mybir.AxisListType.X, op=Alu.max)
                nc.vector.tensor_reduce(
                    mbuf[:, qc:qc + 1], mtmp[:, 0:ntk],
                    axis=mybir.AxisListType.X, op=Alu.max)

            def s_final(bb, h):
                boff = bb * s
                mbuf = mstate.pop((bb, h))
                mps = psst.tile([128, TT], f32, tag="st")
                nc.tensor.transpose(mps[0:n_qc, 0:128], mbuf[:, 0:n_qc],
                                    ident[:, 0:128])
                mrow = mp.tile([n_qc, 128], f32, tag="mrow")
                nc.scalar.copy(mrow[:], mps[0:n_qc, 0:128])
                nc.gpsimd.dma_start(
                    out=qA[h][64:65, boff:boff + s].rearrange(
                        "o (c t) -> o c t", t=128),
                    in_=mrow[:])

            # ================= phase 1: projections ====================
            ph1 = ExitStack()
            xp = ph1.enter_context(tc.tile_pool(name="xp", bufs=2))
            x8p = ph1.enter_context(tc.tile_pool(name="x8p", bufs=2))
            sp1 = ph1.enter_context(tc.tile_pool(name="sp1", bufs=3))
            for tt in range(n_tt):
                tcols = slice(tt * TT, (tt + 1) * TT)
                xt = xp.tile([128, n_dc * TT], f16, tag="x", name="xtile")
                xhr = xh_d.rearrange("(c p) t -> p c t", p=128)
                for ch in range(2):
                    cs = slice(ch * n_dc // 2, (ch + 1) * n_dc // 2)
                    nc.sync.dma_start(
                        out=xt[:].rearrange("p (c t) -> p c t", t=TT)[:, cs],
                        in_=xhr[:, cs, tcols])
                x8 = x8p.tile([128, n_dc, TT], f8, tag="x8", name="x8tile")
                xlr = xl_d.rearrange("(c p) t -> p c t", p=128)
                for ch in range(2):
                    cs = slice(ch * n_dc // 2, (ch + 1) * n_dc // 2)
                    nc.sync.dma_start(
                        out=x8[:, cs], in_=xlr[:, cs, tcols])
                if tt == 0:
                    load_weights_rest()

                def proj(w16, w8, mcol, mwid, ps):
                    for c in range(n_dc):
                        nc.tensor.matmul(
                            ps[:],
                            lhsT=w16[:, c * mwid + mcol:c * mwid + mcol + 128],
                            rhs=xt[:, c * TT:(c + 1) * TT],
                            start=(c == 0), stop=False)
                    for half in range(2):
                        hs = slice(half * 256, half * 256 + 256)
                        for cp in range(n_dc // 2):
                            nc.tensor.matmul(
                                ps[:, hs],
                                lhsT=w8[:, 2 * cp:2 * cp + 2,
                                        mcol:mcol + 128],
                                rhs=x8[:, 2 * cp:2 * cp + 2, hs],
                                start=False,
                                stop=(half == 1 and cp == n_dc // 2 - 1),
                                perf_mode=DR, skip_group_check=True)

                for m in range(QROWS // 128):
                    ps = ps1.tile([128, TT], f32, tag="ps")
                    proj(wq16, wq8, m * 128, QROWS, ps)
                    for i in range(2):
                        h = 2 * m + i
                        rows = slice(i * 64, i * 64 + 64)
                        # qA = fp16(q/8)
                        nc.scalar.activation(qA[h][0:64, tcols], ps[rows, :],
                                             Act.Copy, scale=0.125)
                        # res = q/8 - qA (fp16: keeps gpsimd inputs 16-bit)
                        res = sp1.tile([64, TT], f16, tag="qres")
                        nc.vector.scalar_tensor_tensor(
                            res[:], in0=ps[rows, :], scalar=0.125,
                            in1=qA[h][0:64, tcols],
                            op0=Alu.mult, op1=Alu.subtract)
                        # fp8 pack (SBUF->SBUF on gpsimd)
                        nc.gpsimd.tensor_scalar_mul(
                            qP[h][:, 0, tcols], qA[h][0:64, tcols],
                            1.0 / PK_SCALE)
                        nc.gpsimd.tensor_scalar_mul(
                            qP[h][:, 1, tcols], res[:], PK_SCALE)

                ps = ps1.tile([128, TT], f32, tag="ps")
                proj(wkv16, wkv8, 0, 128, ps)
                nc.scalar.copy(khb[0:64, tcols], ps[0:64, :])
                res = sp1.tile([64, TT], f16, tag="qres")
                nc.vector.scalar_tensor_tensor(
                    res[:], in0=ps[0:64, :], scalar=1.0,
                    in1=khb[0:64, tcols], op0=Alu.mult, op1=Alu.subtract)
                nc.gpsimd.tensor_scalar_mul(
                    kP[:, 0, tcols], res[:], PK_SCALE)
                nc.gpsimd.tensor_scalar_mul(
                    kP[:, 1, tcols], khb[0:64, tcols], 1.0 / PK_SCALE)
                vtmp = sp1.tile([64, TT], f32, tag="vtmp")
                nc.scalar.copy(vtmp[:], ps[64:128, :])
                for j in range(sub):
                    ptr = psst.tile([128, TT], f32, tag="st")
                    nc.tensor.transpose(ptr[0:128, 0:64],
                                        vtmp[:, j * 128:(j + 1) * 128],
                                        ident[0:64, 0:64])
                    ch = tt * sub + j
                    nc.scalar.copy(vhat[:, ch * 65:ch * 65 + 64],
                                   ptr[0:128, 0:64])
                # batch-0 S~ blocks for the q-chunks this tile enabled
                bb, ltt = tt // tt_per_b, tt % tt_per_b
                if bb == 0:
                    for h in range(HEADS_PER_CORE):
                        for qc in range(ltt * sub, (ltt + 1) * sub):
                            s_block(bb, h, qc)
                        if ltt == tt_per_b - 1:
                            s_final(bb, h)
                elif ltt < tt_per_b // 2:
                    # cheap half of batch-1 S~ rides the batch-1 proj tiles
                    for h in range(HEADS_PER_CORE):
                        for qc in range(ltt * sub, (ltt + 1) * sub):
                            s_block(bb, h, qc)

            ph1.close()
            # ============ phase 2 + per-batch o_proj ====================
            load_wo8()
            aop = ph.enter_context(tc.tile_pool(name="aop", bufs=1))
            ptp = ph.enter_context(tc.tile_pool(name="ptp", bufs=6))
            outp = ph.enter_context(tc.tile_pool(name="outp", bufs=3))
            # ao[c]: fp8 pack [128, 2(level), tokens]
            ao = [aop.tile([128, 2, tokens], f8, tag=f"ao{i}", name=f"ao{i}")
                  for i in range(n_oc)]

            def av(pav, pt, lo, w, bb, kc, nchunks):
                ch = bb * (s // 128) + kc
                nc.tensor.matmul(
                    pav[:, lo:lo + w], lhsT=vhat[:, ch * 65:ch * 65 + 65],
                    rhs=pt[:, lo:lo + w],
                    start=(kc == 0), stop=(kc == nchunks - 1),
                    skip_group_check=True)

            b1q = []
            for h in range(HEADS_PER_CORE):
                for qc in range(n_qc // 2, n_qc):
                    b1q.append(lambda h=h, qc=qc: s_block(1, h, qc))
                b1q.append(lambda h=h: s_final(1, h))
            # weight S~ pacing by main-slot size (qt+1 chunks of work);
            # slots run qt-major
            wsum = HEADS_PER_CORE * n_qt * (n_qt + 1) // 2
            bcum, acc = [], 0.0
            for qt in range(n_qt):
                for h in range(HEADS_PER_CORE):
                    acc += (qt + 1) * len(b1q) / wsum
                    bcum.append(min(int(round(acc)), len(b1q)))
            bcum[-1] = len(b1q)

            out_r = out_d.rearrange("(mm p) t -> p mm t", p=128)

            def oproj_group(bb, qt, g, eng):
                """4 consecutive m-blocks of one q-tile, one DMA out."""
                boff = bb * s
                osb = outp.tile([128, 4, TT], f16, tag="og", name="osbg")
                for mi in range(4):
                    m = g * 4 + mi
                    po = ps1.tile([128, TT], f32, tag="ps")
                    for half in range(2):
                        hs = slice(half * 256, half * 256 + 256)
                        qsl = slice(boff + qt * TT + half * 256,
                                    boff + qt * TT + half * 256 + 256)
                        for ci in range(n_oc):
                            nc.tensor.matmul(
                                po[:, hs],
                                lhsT=wo8[ci][:, :, m * 128:m * 128 + 128],
                                rhs=ao[ci][:, :, qsl],
                                start=(ci == 0), stop=(ci == n_oc - 1),
                                perf_mode=DR,
                                skip_group_check=(half == 1))
                    dst = osb[:, mi, :]
                    if eng == "v":
                        nc.vector.tensor_copy(dst, po[:])
                    else:
                        nc.scalar.copy(dst, po[:])
                nc.sync.dma_start(
                    out=out_r[:, g * 4:g * 4 + 4,
                              boff + qt * TT:boff + (qt + 1) * TT],
                    in_=osb[:])

            def attn_slot(bb, h, qt):
                boff = bb * s
                qlo = boff + qt * TT
                pav = psav.tile([65, TT], f32, tag="pav")
                nchunks = (qt + 1) * sub
                pipe = []
                for kc in range(nchunks):
                    ksl = slice(boff + kc * 128, boff + kc * 128 + 128)
                    j = kc - qt * sub
                    lo = max(j, 0) * 128  # cols < lo fully masked
                    w = TT - lo
                    s2 = ps1.tile([128, TT], f32, tag="ps")
                    nc.tensor.matmul(
                        s2[:, lo:lo + w], lhsT=khb[:, ksl],
                        rhs=qA[h][:, qlo + lo:qlo + TT],
                        start=True, stop=False)
                    # fp8 DoubleRow lo-pass (<=256-wide halves)
                    nhalf = (w + 255) // 256
                    for hf in range(nhalf):
                        hlo = lo + hf * 256
                        hw = min(256, TT - hlo)
                        nc.tensor.matmul(
                            s2[:, hlo:hlo + hw],
                            lhsT=kP[:, :, ksl],
                            rhs=qP[h][:, :, qlo + hlo:qlo + hlo + hw],
                            start=False,
                            stop=(j < 0 and hf == nhalf - 1),
                            perf_mode=DR, skip_group_check=True)
                    if j >= 0:
                        nc.tensor.matmul(
                            s2[:, lo:lo + 128], lhsT=identM[:],
                            rhs=maskKQ[:], start=False, stop=True,
                            skip_group_check=True)
                    pt = ptp.tile([128, TT], f32r, tag="pt")
                    nc.scalar.activation(pt[:, lo:lo + w],
                                         s2[:, lo:lo + w], Act.Exp)
                    pipe.append((pt, lo, w, kc))
                    if len(pipe) > 2:
                        pv = pipe.pop(0)
                        av(pav, pv[0], pv[1], pv[2], bb, pv[3], nchunks)
                for pv in pipe:
                    av(pav, pv[0], pv[1], pv[2], bb, pv[3], nchunks)
                pipe.clear()

                # pav row 64 = l/4; rec = 4/l; broadcast on gpsimd; then
                # aof = PV * 4/l = 4*attn (fp16). AO0 = fp8(aof/16)
                # = fp8(attn/4), AO1 = fp8(aof/16 - AO0) = attn/4 - AO0.
                # Both wo8 levels are 4*ternary.
                # Pool only sees fp16->fp8 ops; the mixed stt runs on DVE.
                rec = mp.tile([1, TT], f32, tag="rec")
                with nc.allow_low_precision(
                        reason="1/l broadcast feeds fp8 conversions"):
                    nc.vector.reciprocal(rec[:], pav[64:65, :])
                bcs = mp.tile([64, TT], f32, tag="bcs")
                nc.gpsimd.partition_broadcast(bcs[:], rec[:])
                # aof half matches ao's base partition (SB+SB ops
                # require equal base partitions)
                rows = slice((h % 2) * 64, (h % 2) * 64 + 64)
                aof = mp.tile([128, TT], f16, tag="aof")
                nc.vector.tensor_tensor(
                    aof[rows, :], pav[0:64, :], bcs[:], op=Alu.mult)
                c = h // 2
                nc.gpsimd.tensor_scalar_mul(
                    ao[c][rows, 0, qlo:qlo + TT], aof[rows, :], 1.0 / 16)
                nc.vector.scalar_tensor_tensor(
                    ao[c][rows, 1, qlo:qlo + TT],
                    in0=aof[rows, :], scalar=1.0 / 16,
                    in1=ao[c][rows, 0, qlo:qlo + TT],
                    op0=Alu.mult, op1=Alu.subtract)

            # Both batches qt-major: each q-tile's o_proj rides the next
            # q-tile's attention slots. Batch-0 slots also carry the
            # batch-1 S~ rideshare; batch-0's o_proj copies go to ScalarE
            # (DVE is reduce-bound there), batch-1's to DVE (ScalarE is
            # exp-bound there).
            slot = 0
            for qt in range(n_qt):
                for h in range(HEADS_PER_CORE):
                    attn_slot(0, h, qt)
                    lo_i = bcum[slot - 1] if slot else 0
                    for fn in b1q[lo_i:bcum[slot]]:
                        fn()
                    slot += 1
                    if qt > 0:
                        oproj_group(0, qt - 1, h, "s")
            # pending o_proj groups pop one per slot, delayed one slot so
            # the group's ao dependencies never head-block the PE queue
            pend = [(0, n_qt - 1, g, "s") for g in range(HEADS_PER_CORE)]
            for qt in range(n_qt):
                for h in range(HEADS_PER_CORE):
                    attn_slot(1, h, qt)
                    if qt > 0 or h > 0:
                        oproj_group(*pend.pop(0))
                pend += [(1, qt, g, "v" if g % 2 else "s")
                         for g in range(HEADS_PER_CORE)]
            for args in pend:
                oproj_group(*args)

    nc.compile()
    return nc


def _ternarize(w):
    w = np.asarray(w, np.float32)
    scale = max(np.abs(w).mean(), 1e-6)
    return ((w > 0.05 * scale).astype(np.float32)
            - (w < -0.05 * scale).astype(np.float32))


def kernel(x, wq, wk, wv, wo):
    from concourse.bass_utils import run_bass_kernel_spmd

    if "nc" not in _CACHE:
        _CACHE["nc"] = _build_program()
    nc = _CACHE["nc"]

    tq = _ternarize(wq)
    tk = _ternarize(wk)
    tv = _ternarize(wv)
    to = _ternarize(wo)

    xT = np.ascontiguousarray(np.asarray(x, np.float32).reshape(B * S, D).T)
    xh = xT.astype(np.float16)
    xl = ((xT - xh.astype(np.float32)) * XL_SCALE).astype(E4)

    in_maps = []
    for c in range(NCORES):
        qsl = slice(c * QROWS, (c + 1) * QROWS)
        ksl = slice(c * HD, (c + 1) * HD)
        wkv = np.concatenate([tk[ksl], tv[ksl]], axis=0)  # [128, D]
        wqT = np.ascontiguousarray(tq[qsl].T)
        wkvT = np.ascontiguousarray(wkv.T)
        in_maps.append({
            "xh": xh, "xl": xl,
            "wq_hi": wqT.astype(np.float16),
            "wq_lo": (wqT / XL_SCALE).astype(E4),
            "wkv_hi": wkvT.astype(np.float16),
            "wkv_lo": (wkvT / XL_SCALE).astype(E4),
            "wo8": np.ascontiguousarray(to[:, qsl].T * 4.0).astype(E4),
        })

    res = run_bass_kernel_spmd(nc, in_maps, list(range(NCORES)))
    total = res.results[0]["out"].astype(np.float32)
    for c in range(1, NCORES):
        total = total + res.results[c]["out"].astype(np.float32)
    return np.ascontiguousarray(total.T).reshape(B, S, D).astype(np.float32)


# revision 64
# speedup vs baseline: 1.4734x; 1.0150x over previous
"""GQA causal attention (ternary weights) on 8 TRN2 NeuronCores.

Strategy (tensor-parallel over heads, per sharding hint):
  - core c owns Q heads [4c, 4c+4) and KV head c.
  - host: ternarize weights; split x into fp16 hi + fp8(e4m3, x512) residual;
    fp8 weight copies are ternary/512 (exact: 2^-9 is an e4m3 subnormal) so
    the fp8 DoubleRow lo-pass lands at natural scale in the same PSUM group
    as the fp16 hi-pass.
  - device per core:
      phase 1: q/k/v projections: fp16 hi matmuls + fp8 DoubleRow lo matmuls
               (2 contraction chunks per instruction, half cost). Activations
               are stored as fp16 (qA rows 0:64 = q/8, khb rows 0:64 = k,
               row 64 = bias) plus fp8 packs for the attention lo-pass:
               q-pack = (qh/64, qres*64), k-pack = (kres*64, kh/64).
      phase 2: per (batch, head): fp16 S~=QK^T in [q,k] layout for row-max
               (diagonal causal mask folded in as an identity x mask-const
               matmul on the PE); exact S^T in [k,q] via fp16 hi matmul
               (with folded -max bias row) + one fp8 DoubleRow lo matmul;
               exp on ScalarE; PV + row-sums via fp32r matmul with a 0.25
               column appended to V (so the normalizer is 1/(4l)).
      phase 3: o_proj via 2-level fp8 DoubleRow: AO0=fp8(ao/4),
               AO1=fp8(ao/4-AO0), weights 4*ternary (exact in fp8); fp16
               partial outputs DMA'd per batch so they overlap attention.
  - host: sum the 8 fp16 partial outputs in fp32 (row-split "all-reduce").
"""

import sys

sys.path.insert(0, "/opt/trn_rl_repo")

import numpy as np
import ml_dtypes

B = 2
S = 2048
D = 2048
NCORES = 8
HEADS_PER_CORE = 4
HD = 64
QROWS = HEADS_PER_CORE * HD  # 256
TT = 512  # token tile
MASK_NEG = -30000.0
XL_SCALE = 512.0     # x residual fp8 scale (weights get 1/512)
PK_SCALE = 64.0      # S lo-pass pack scale (carrier /64, residual x64)

E4 = ml_dtypes.float8_e4m3

_CACHE = {}


def _build_program(b=B, s=S, d=D):
    import concourse.bacc as bacc
    import concourse.tile as tile
    import concourse.mybir as mybir
    from concourse import masks
    from contextlib import ExitStack

    f32 = mybir.dt.float32
    f32r = mybir.dt.float32r
    f16 = mybir.dt.float16
    f8 = mybir.dt.float8e4
    Alu = mybir.AluOpType
    Act = mybir.ActivationFunctionType
    DR = mybir.MatmulPerfMode.DoubleRow

    tokens = b * s
    n_tt = tokens // TT          # token tiles
    tt_per_b = s // TT
    n_dc = d // 128              # contraction chunks for projections
    n_qt = s // TT               # 512-wide q tiles per batch
    n_qc = s // 128              # 128-wide q chunks per batch (max pass)
    n_mt = d // 128              # output row tiles for o_proj
    n_oc = QROWS // 128          # o_proj contraction chunks (2)
    sub = TT // 128              # 128-sub-blocks per 512 tile (4)

    nc = bacc.Bacc("TRN2", target_bir_lowering=False, debug=False,
                   num_devices=NCORES)

    x_d = [nc.dram_tensor(f"x{i}", [d, tokens], f8,
                          kind="ExternalInput").ap() for i in range(3)]
    wq_d = [nc.dram_tensor(f"wq{i}", [d, QROWS], f8,
                           kind="ExternalInput").ap() for i in range(3)]
    wkv_d = [nc.dram_tensor(f"wkv{i}", [d, 128], f8,
                            kind="ExternalInput").ap() for i in range(3)]
    wo_d = nc.dram_tensor("wo8", [QROWS, d], f8, kind="ExternalInput").ap()
    out_d = nc.dram_tensor("out", [d, tokens], f16, kind="ExternalOutput").ap()

    with tile.TileContext(nc) as tc, ExitStack() as top:
        constp = top.enter_context(tc.tile_pool(name="const", bufs=1))
        wpool = top.enter_context(tc.tile_pool(name="wts", bufs=1))
        pp = top.enter_context(tc.tile_pool(name="persist", bufs=1))

        # --- constants -------------------------------------------------
        # maskKQ[p,q] = MASK_NEG where p > q (keep k<=q in [k,q] layout)
        maskKQ = constp.tile([128, 128], f16, tag="maskKQ")
        nc.gpsimd.memset(maskKQ[:], 0.0)
        nc.gpsimd.affine_select(
            out=maskKQ[:], in_=maskKQ[:], compare_op=Alu.is_ge, fill=MASK_NEG,
            base=0, pattern=[[1, 128]], channel_multiplier=-1)
        # maskQK[q,k] = MASK_NEG where k > q ([q,k] layout)
        maskQK = constp.tile([128, 128], f16, tag="maskQK")
        nc.gpsimd.memset(maskQK[:], 0.0)
        nc.gpsimd.affine_select(
            out=maskQK[:], in_=maskQK[:], compare_op=Alu.is_ge, fill=MASK_NEG,
            base=0, pattern=[[-1, 128]], channel_multiplier=1)
        identM = constp.tile([128, 128], f16, tag="identM")
        masks.make_identity(nc, identM[:])
        ident = constp.tile([128, 128], f32, tag="ident")
        masks.make_identity(nc, ident[:])


        # --- weights (DMA order tuned so tile-0 compute starts early) ---
        wq8 = [wpool.tile([128, n_dc, QROWS], f8, tag=f"wq8_{i}",
                          name=f"wq8_{i}") for i in range(3)]
        wkv8 = [wpool.tile([128, n_dc, 128], f8, tag=f"wkv8_{i}",
                           name=f"wkv8_{i}") for i in range(3)]
        nc.sync.dma_start(
            out=wq8[0][:, :, 0:128],
            in_=wq_d[0].rearrange("(c p) n -> p c n", p=128)[:, :, 0:128])

        def load_weights_rest():
            nc.sync.dma_start(
                out=wq8[0][:, :, 128:QROWS],
                in_=wq_d[0].rearrange("(c p) n -> p c n",
                                      p=128)[:, :, 128:QROWS])
            for i in range(3):
                if i > 0:
                    nc.sync.dma_start(
                        out=wq8[i][:],
                        in_=wq_d[i].rearrange("(c p) n -> p c n", p=128))
                nc.sync.dma_start(
                    out=wkv8[i][:],
                    in_=wkv_d[i].rearrange("(c p) n -> p c n", p=128))

        # o_proj weights: [128, 2(level), d] per contraction chunk; both
        # levels are the same 4*ternary data. Loaded at end of phase 1.
        wo8 = [wpool.tile([128, 2, d], f8, tag=f"wo8_{c}", name=f"wo8_{c}")
               for c in range(n_oc)]

        def load_wo8():
            for c in range(n_oc):
                for lv in range(2):
                    nc.sync.dma_start(
                        out=wo8[c][:, lv, :],
                        in_=wo_d[c * 128:(c + 1) * 128, :])

        # --- persistent activations -----------------------------------
        # qA[h]: rows 0:64 = fp16(q/8), row 64 = m~ bias (max pass)
        # qP[h]: fp8 pack [64, 2, tokens] = (qh/64, qres*64)
        # khb:   rows 0:64 = fp16(k), row 64 = -1
        # kP:    fp8 pack [64, 2, tokens] = (kres*64, kh/64)
        # vhat:  [128, chunk*65]: cols 0:64 of chunk = v, col 64 = 0.25
        qA = [pp.tile([65, tokens], f16, tag=f"qA{h}", name=f"qA{h}")
              for h in range(HEADS_PER_CORE)]
        qP = [pp.tile([64, 2, tokens], f8, tag=f"qP{h}", name=f"qP{h}")
              for h in range(HEADS_PER_CORE)]
        khb = pp.tile([65, tokens], f16, tag="khb")
        kP = pp.tile([64, 2, tokens], f8, tag="kP")
        n_ch = tokens // 128
        vhat = pp.tile([128, n_ch * 65], f32r, tag="vhat")
        nc.scalar.activation(
            vhat[:], ident[:, 0:1].to_broadcast([128, n_ch * 65]),
            Act.Copy, bias=0.25, scale=0.0)
        nc.gpsimd.memset(khb[64:65, :], -1.0)

        with ExitStack() as ph:
            mp = ph.enter_context(tc.tile_pool(name="mp", bufs=3))
            ps1 = ph.enter_context(
                tc.tile_pool(name="ps1", bufs=5, space="PSUM"))
            psst = ph.enter_context(
                tc.tile_pool(name="psst", bufs=2, space="PSUM"))
            psav = ph.enter_context(
                tc.tile_pool(name="psav", bufs=1, space="PSUM"))

            # ---------- S~ max-estimate pass, as schedulable blocks ------
            mstate = {}
            mbp = ph.enter_context(tc.tile_pool(name="mbp", bufs=8))

            def s_block(bb, h, qc):
                boff = bb * s
                if qc == 0:
                    mstate[(bb, h)] = mbp.tile([128, n_qc], f32, tag="mbuf",
                                               name="mbuf")
                mbuf = mstate[(bb, h)]
                qsl = slice(boff + qc * 128, boff + qc * 128 + 128)
                ntk = qc // sub + 1
                mtmp = mp.tile([128, 8], f32, tag="mtmp")
                for kt in range(ntk):
                    w = min(TT, (qc + 1) * 128 - kt * TT)
                    st = psst.tile([128, TT], f32, tag="st")
                    nc.tensor.matmul(
                        st[:, 0:w],
                        lhsT=qA[h][0:64, qsl],
                        rhs=khb[0:64, boff + kt * TT:boff + kt * TT + w],
                        start=True, stop=(kt != ntk - 1))
                    if kt == ntk - 1:  # diagonal block is last 128 cols
                        nc.tensor.matmul(
                            st[:, w - 128:w], lhsT=identM[:], rhs=maskQK[:],
                            start=False, stop=True, skip_group_check=True)
                    nc.vector.tensor_reduce(
                        mtmp[:, kt:kt + 1], st[:, 0:w],
                        axis=mybir.AxisListType.X, op=Alu.max)
                nc.vector.tensor_reduce(
                    mbuf[:, qc:qc + 1], mtmp[:, 0:ntk],
                    axis=mybir.AxisListType.X, op=Alu.max)

            def s_final(bb, h):
                boff = bb * s
                mbuf = mstate.pop((bb, h))
                mps = psst.tile([128, TT], f32, tag="st")
                nc.tensor.transpose(mps[0:n_qc, 0:128], mbuf[:, 0:n_qc],
                                    ident[:, 0:128])
                mrow = mp.tile([n_qc, 128], f32, tag="mrow")
                nc.scalar.copy(mrow[:], mps[0:n_qc, 0:128])
                nc.gpsimd.dma_start(
                    out=qA[h][64:65, boff:boff + s].rearrange(
                        "o (c t) -> o c t", t=128),
                    in_=mrow[:])

            # ================= phase 1: projections ====================
            ph1 = ExitStack()
            xp = ph1.enter_context(tc.tile_pool(name="xp", bufs=2))
            x8p = ph1.enter_context(tc.tile_pool(name="x8p", bufs=2))
            sp1 = ph1.enter_context(tc.tile_pool(name="sp1", bufs=3))
            for tt in range(n_tt):
                tcols = slice(tt * TT, (tt + 1) * TT)
                x8 = [x8p.tile([128, n_dc, TT], f8, tag=f"x8_{i}",
                               name=f"x8_{i}") for i in range(3)]
                for i in range(3):
                    xr = x_d[i].rearrange("(c p) t -> p c t", p=128)
                    for ch in range(2):
                        cs = slice(ch * n_dc // 2, (ch + 1) * n_dc // 2)
                        nc.sync.dma_start(
                            out=x8[i][:, cs], in_=xr[:, cs, tcols])
                if tt == 0:
                    load_weights_rest()

                def proj(w8l, mcol, ps):
                    # each 256-wide half is its own accumulation group
                    for half in range(2):
                        hs = slice(half * 256, half * 256 + 256)
                        for lv in range(3):
                            for cp in range(n_dc // 2):
                                nc.tensor.matmul(
                                    ps[:, hs],
                                    lhsT=w8l[lv][:, 2 * cp:2 * cp + 2,
                                                 mcol:mcol + 128],
                                    rhs=x8[lv][:, 2 * cp:2 * cp + 2, hs],
                                    start=(lv == 0 and cp == 0),
                                    stop=(lv == 2 and cp == n_dc // 2 - 1),
                                    perf_mode=DR,
                                    skip_group_check=(half == 1))
                                first = False

                for m in range(QROWS // 128):
                    ps = ps1.tile([128, TT], f32, tag="ps")
                    proj(wq8, m * 128, ps)
                    for i in range(2):
                        h = 2 * m + i
                        rows = slice(i * 64, i * 64 + 64)
                        # qA = fp16(q/8)
                        nc.scalar.activation(qA[h][0:64, tcols], ps[rows, :],
                                             Act.Copy, scale=0.125)
                        # res = q/8 - qA (fp16: keeps gpsimd inputs 16-bit)
                        res = sp1.tile([64, TT], f16, tag="qres")
                        nc.vector.scalar_tensor_tensor(
                            res[:], in0=ps[rows, :], scalar=0.125,
                            in1=qA[h][0:64, tcols],
                            op0=Alu.mult, op1=Alu.subtract)
                        # fp8 pack (SBUF->SBUF on gpsimd)
                        nc.gpsimd.tensor_scalar_mul(
                            qP[h][:, 0, tcols], qA[h][0:64, tcols],
                            1.0 / PK_SCALE)
                        nc.gpsimd.tensor_scalar_mul(
                            qP[h][:, 1, tcols], res[:], PK_SCALE)

                ps = ps1.tile([128, TT], f32, tag="ps")
                proj(wkv8, 0, ps)
                nc.scalar.copy(khb[0:64, tcols], ps[0:64, :])
                res = sp1.tile([64, TT], f16, tag="qres")
                nc.vector.scalar_tensor_tensor(
                    res[:], in0=ps[0:64, :], scalar=1.0,
                    in1=khb[0:64, tcols], op0=Alu.mult, op1=Alu.subtract)
                nc.gpsimd.tensor_scalar_mul(
                    kP[:, 0, tcols], res[:], PK_SCALE)
                nc.gpsimd.tensor_scalar_mul(
                    kP[:, 1, tcols], khb[0:64, tcols], 1.0 / PK_SCALE)
                vtmp = sp1.tile([64, TT], f32, tag="vtmp")
                nc.scalar.copy(vtmp[:], ps[64:128, :])
                for j in range(sub):
                    ptr = psst.tile([128, TT], f32, tag="st")
                    nc.tensor.transpose(ptr[0:128, 0:64],
                                        vtmp[:, j * 128:(j + 1) * 128],
                                        ident[0:64, 0:64])
                    ch = tt * sub + j
                    nc.scalar.copy(vhat[:, ch * 65:ch * 65 + 64],
                                   ptr[0:128, 0:64])
                # batch-0 S~ blocks for the q-chunks this tile enabled
                bb, ltt = tt // tt_per_b, tt % tt_per_b
                if bb == 0:
                    for h in range(HEADS_PER_CORE):
                        for qc in range(ltt * sub, (ltt + 1) * sub):
                            s_block(bb, h, qc)
                        if ltt == tt_per_b - 1:
                            s_final(bb, h)
                elif ltt < tt_per_b // 2:
                    # cheap half of batch-1 S~ rides the batch-1 proj tiles
                    for h in range(HEADS_PER_CORE):
                        for qc in range(ltt * sub, (ltt + 1) * sub):
                            s_block(bb, h, qc)

            ph1.close()
            # ============ phase 2 + per-batch o_proj ====================
            load_wo8()
            aop = ph.enter_context(tc.tile_pool(name="aop", bufs=1))
            ptp = ph.enter_context(tc.tile_pool(name="ptp", bufs=6))
            outp = ph.enter_context(tc.tile_pool(name="outp", bufs=3))
            # ao[c]: fp8 pack [128, 2(level), tokens]
            ao = [aop.tile([128, 2, tokens], f8, tag=f"ao{i}", name=f"ao{i}")
                  for i in range(n_oc)]

            def av(pav, pt, lo, w, bb, kc, nchunks):
                ch = bb * (s // 128) + kc
                nc.tensor.matmul(
                    pav[:, lo:lo + w], lhsT=vhat[:, ch * 65:ch * 65 + 65],
                    rhs=pt[:, lo:lo + w],
                    start=(kc == 0), stop=(kc == nchunks - 1),
                    skip_group_check=True)

            b1q = []
            for h in range(HEADS_PER_CORE):
                for qc in range(n_qc // 2, n_qc):
                    b1q.append(lambda h=h, qc=qc: s_block(1, h, qc))
                b1q.append(lambda h=h: s_final(1, h))
            # weight S~ pacing by main-slot size (qt+1 chunks of work);
            # slots run qt-major
            wsum = HEADS_PER_CORE * n_qt * (n_qt + 1) // 2
            bcum, acc = [], 0.0
            for qt in range(n_qt):
                for h in range(HEADS_PER_CORE):
                    acc += (qt + 1) * len(b1q) / wsum
                    bcum.append(min(int(round(acc)), len(b1q)))
            bcum[-1] = len(b1q)

            out_r = out_d.rearrange("(mm p) t -> p mm t", p=128)

            def oproj_group(bb, qt, g, eng):
                """4 consecutive m-blocks of one q-tile, one DMA out."""
                boff = bb * s
                osb = outp.tile([128, 4, TT], f16, tag="og", name="osbg")
                for mi in range(4):
                    m = g * 4 + mi
                    po = ps1.tile([128, TT], f32, tag="ps")
                    for half in range(2):
                        hs = slice(half * 256, half * 256 + 256)
                        qsl = slice(boff + qt * TT + half * 256,
                                    boff + qt * TT + half * 256 + 256)
                        for ci in range(n_oc):
                            nc.tensor.matmul(
                                po[:, hs],
                                lhsT=wo8[ci][:, :, m * 128:m * 128 + 128],
                                rhs=ao[ci][:, :, qsl],
                                start=(ci == 0), stop=(ci == n_oc - 1),
                                perf_mode=DR,
                                skip_group_check=(half == 1))
                    dst = osb[:, mi, :]
                    if eng == "v":
                        nc.vector.tensor_copy(dst, po[:])
                    else:
                        nc.scalar.copy(dst, po[:])
                nc.sync.dma_start(
                    out=out_r[:, g * 4:g * 4 + 4,
                              boff + qt * TT:boff + (qt + 1) * TT],
                    in_=osb[:])

            def attn_slot(bb, h, qt):
                boff = bb * s
                qlo = boff + qt * TT
                pav = psav.tile([65, TT], f32, tag="pav")
                nchunks = (qt + 1) * sub
                pipe = []
                for kc in range(nchunks):
                    ksl = slice(boff + kc * 128, boff + kc * 128 + 128)
                    j = kc - qt * sub
                    lo = max(j, 0) * 128  # cols < lo fully masked
                    w = TT - lo
                    s2 = ps1.tile([128, TT], f32, tag="ps")
                    nc.tensor.matmul(
                        s2[:, lo:lo + w], lhsT=khb[:, ksl],
                        rhs=qA[h][:, qlo + lo:qlo + TT],
                        start=True, stop=False)
                    # fp8 DoubleRow lo-pass (<=256-wide halves)
                    nhalf = (w + 255) // 256
                    for hf in range(nhalf):
                        hlo = lo + hf * 256
                        hw = min(256, TT - hlo)
                        nc.tensor.matmul(
                            s2[:, hlo:hlo + hw],
                            lhsT=kP[:, :, ksl],
                            rhs=qP[h][:, :, qlo + hlo:qlo + hlo + hw],
                            start=False,
                            stop=(j < 0 and hf == nhalf - 1),
                            perf_mode=DR, skip_group_check=True)
                    if j >= 0:
                        nc.tensor.matmul(
                            s2[:, lo:lo + 128], lhsT=identM[:],
                            rhs=maskKQ[:], start=False, stop=True,
                            skip_group_check=True)
                    pt = ptp.tile([128, TT], f32r, tag="pt")
                    nc.scalar.activation(pt[:, lo:lo + w],
                                         s2[:, lo:lo + w], Act.Exp)
                    pipe.append((pt, lo, w, kc))
                    if len(pipe) > 3:
                        pv = pipe.pop(0)
                        av(pav, pv[0], pv[1], pv[2], bb, pv[3], nchunks)
                for pv in pipe:
                    av(pav, pv[0], pv[1], pv[2], bb, pv[3], nchunks)
                pipe.clear()

                # pav row 64 = l/4; rec = 4/l; broadcast on gpsimd; then
                # aof = PV * 4/l = 4*attn (fp16). AO0 = fp8(aof/16)
                # = fp8(attn/4), AO1 = fp8(aof/16 - AO0) = attn/4 - AO0.
                # Both wo8 levels are 4*ternary.
                # Pool only sees fp16->fp8 ops; the mixed stt runs on DVE.
                rec = mp.tile([1, TT], f32, tag="rec")
                with nc.allow_low_precision(
                        reason="1/l broadcast feeds fp8 conversions"):
                    nc.vector.reciprocal(rec[:], pav[64:65, :])
                bcs = mp.tile([64, TT], f32, tag="bcs")
                nc.gpsimd.partition_broadcast(bcs[:], rec[:])
                # aof half matches ao's base partition (SB+SB ops
                # require equal base partitions)
                rows = slice((h % 2) * 64, (h % 2) * 64 + 64)
                aof = mp.tile([128, TT], f16, tag="aof")
                nc.vector.tensor_tensor(
                    aof[rows, :], pav[0:64, :], bcs[:], op=Alu.mult)
                c = h // 2
                nc.gpsimd.tensor_scalar_mul(
                    ao[c][rows, 0, qlo:qlo + TT], aof[rows, :], 1.0 / 16)
                nc.vector.scalar_tensor_tensor(
                    ao[c][rows, 1, qlo:qlo + TT],
                    in0=aof[rows, :], scalar=1.0 / 16,
                    in1=ao[c][rows, 0, qlo:qlo + TT],
                    op0=Alu.mult, op1=Alu.subtract)

            # Both batches qt-major: each q-tile's o_proj rides the next
            # q-tile's attention slots. Batch-0 slots also carry the
            # batch-1 S~ rideshare; batch-0's o_proj copies go to ScalarE
            # (DVE is reduce-bound there), batch-1's to DVE (ScalarE is
            # exp-bound there).
            slot = 0
            for qt in range(n_qt):
                for h in range(HEADS_PER_CORE):
                    attn_slot(0, h, qt)
                    lo_i = bcum[slot - 1] if slot else 0
                    for fn in b1q[lo_i:bcum[slot]]:
                        fn()
                    slot += 1
                    if qt > 0:
                        oproj_group(0, qt - 1, h, "s")
            # pending o_proj groups pop one per slot, delayed one slot so
            # the group's ao dependencies never head-block the PE queue
            pend = [(0, n_qt - 1, g, "v") for g in range(HEADS_PER_CORE)]
            for qt in range(n_qt):
                for h in range(HEADS_PER_CORE):
                    attn_slot(1, h, qt)
                    if qt > 0 or h > 0:
                        oproj_group(*pend.pop(0))
                pend += [(1, qt, g, "v") for g in range(HEADS_PER_CORE)]
            for args in pend:
                oproj_group(*args)

    nc.compile()
    return nc


def _ternarize(w):
    w = np.asarray(w, np.float32)
    scale = max(np.abs(w).mean(), 1e-6)
    return ((w > 0.05 * scale).astype(np.float32)
            - (w < -0.05 * scale).astype(np.float32))


def kernel(x, wq, wk, wv, wo):
    from concourse.bass_utils import run_bass_kernel_spmd

    if "nc" not in _CACHE:
        _CACHE["nc"] = _build_program()
    nc = _CACHE["nc"]

    tq = _ternarize(wq)
    tk = _ternarize(wk)
    tv = _ternarize(wv)
    to = _ternarize(wo)

    xT = np.ascontiguousarray(np.asarray(x, np.float32).reshape(B * S, D).T)
    # 3-level e4m3 split of x; weight copies at 1, 1/32, 1/512 (all exact)
    x0 = xT.astype(E4)
    r1 = xT - x0.astype(np.float32)
    x1 = (r1 * 32.0).astype(E4)
    r2 = r1 - x1.astype(np.float32) / 32.0
    x2 = (r2 * 512.0).astype(E4)
    xs = [x0, x1, x2]
    scales = [1.0, 1.0 / 32, 1.0 / 512]

    in_maps = []
    for c in range(NCORES):
        qsl = slice(c * QROWS, (c + 1) * QROWS)
        ksl = slice(c * HD, (c + 1) * HD)
        wkv = np.concatenate([tk[ksl], tv[ksl]], axis=0)  # [128, D]
        wqT = np.ascontiguousarray(tq[qsl].T)
        wkvT = np.ascontiguousarray(wkv.T)
        m = {"wo8": np.ascontiguousarray(to[:, qsl].T * 4.0).astype(E4)}
        for i in range(3):
            m[f"x{i}"] = xs[i]
            m[f"wq{i}"] = (wqT * scales[i]).astype(E4)
            m[f"wkv{i}"] = (wkvT * scales[i]).astype(E4)
        in_maps.append(m)

    res = run_bass_kernel_spmd(nc, in_maps, list(range(NCORES)))
    total = res.results[0]["out"].astype(np.float32)
    for c in range(1, NCORES):
        total = total + res.results[c]["out"].astype(np.float32)
    return np.ascontiguousarray(total.T).reshape(B, S, D).astype(np.float32)


# revision 66
# speedup vs baseline: 1.4769x; 1.0024x over previous
"""GQA causal attention (ternary weights) on 8 TRN2 NeuronCores.

Strategy (tensor-parallel over heads, per sharding hint):
  - core c owns Q heads [4c, 4c+4) and KV head c.
  - host: ternarize weights; split x into THREE e4m3 fp8 levels
    (x0=fp8(x), x1=fp8(res*32), x2=fp8(res2*512)); matching fp8 weight
    copies at 1, 1/32, 1/512 (all exact e4m3 values, 2^-9 is a subnormal),
    so every projection pass is an fp8 DoubleRow matmul at natural scale.
  - device per core:
      phase 1: q/k/v projections as 3x fp8 DoubleRow passes (2 contraction
               chunks per instruction at half cost = 4x fp16 throughput)
               accumulated in fp32 PSUM. Stored as fp16 qA (q/8, row 64 =
               m~ bias), fp16 khb (k, row 64 = -1), and fp8 packs for the
               attention lo-pass: q-pack (qh/64, qres*64), k-pack
               (kres*64, kh/64). Batch-0 S~ max-pass rides these tiles;
               the cheap half of batch-1's S~ rides batch-1's proj tiles.
      phase 2: per (batch, head, q-tile): fp16 S~=QK^T in [q,k] layout for
               the row max (causal mask folded in as identity x mask-const
               matmuls on the PE; maxes via DVE X-reduce); exact S^T in
               [k,q] = fp16 hi matmul (with folded -max bias row) + one
               fp8 DoubleRow lo matmul; exp on ScalarE; PV + row sums via
               a single fp32r matmul with a 0.25 column appended to V.
               1/l via DVE reciprocal + gpsimd partition_broadcast.
      phase 3: o_proj via 2-level fp8 DoubleRow (AO0=fp8(attn/4),
               AO1=fp8(attn/4-AO0), weights 4*ternary, exact in fp8);
               fp16 partial outputs, one grouped DMA per 4 row-blocks,
               interleaved into the following q-tile's attention slots.
  - host: sum the 8 fp16 partial outputs in fp32 (row-split "all-reduce").
"""

import sys

sys.path.insert(0, "/opt/trn_rl_repo")

import numpy as np
import ml_dtypes

B = 2
S = 2048
D = 2048
NCORES = 8
HEADS_PER_CORE = 4
HD = 64
QROWS = HEADS_PER_CORE * HD  # 256
TT = 512  # token tile
MASK_NEG = -30000.0
PK_SCALE = 64.0      # S lo-pass pack scale (carrier /64, residual x64)

E4 = ml_dtypes.float8_e4m3

_CACHE = {}


def _build_program(b=B, s=S, d=D):
    import concourse.bacc as bacc
    import concourse.tile as tile
    import concourse.mybir as mybir
    from concourse import masks
    from contextlib import ExitStack

    f32 = mybir.dt.float32
    f32r = mybir.dt.float32r
    f16 = mybir.dt.float16
    f8 = mybir.dt.float8e4
    Alu = mybir.AluOpType
    Act = mybir.ActivationFunctionType
    DR = mybir.MatmulPerfMode.DoubleRow

    tokens = b * s
    n_tt = tokens // TT          # token tiles
    tt_per_b = s // TT
    n_dc = d // 128              # contraction chunks for projections
    n_qt = s // TT               # 512-wide q tiles per batch
    n_qc = s // 128              # 128-wide q chunks per batch (max pass)
    n_mt = d // 128              # output row tiles for o_proj
    n_oc = QROWS // 128          # o_proj contraction chunks (2)
    sub = TT // 128              # 128-sub-blocks per 512 tile (4)

    nc = bacc.Bacc("TRN2", target_bir_lowering=False, debug=False,
                   num_devices=NCORES)

    x_d = [nc.dram_tensor(f"x{i}", [d, tokens], f8,
                          kind="ExternalInput").ap() for i in range(3)]
    wq_d = [nc.dram_tensor(f"wq{i}", [d, QROWS], f8,
                           kind="ExternalInput").ap() for i in range(3)]
    wkv_d = [nc.dram_tensor(f"wkv{i}", [d, 128], f8,
                            kind="ExternalInput").ap() for i in range(3)]
    wo_d = nc.dram_tensor("wo8", [QROWS, d], f8, kind="ExternalInput").ap()
    out_d = nc.dram_tensor("out", [d, tokens], f16, kind="ExternalOutput").ap()

    with tile.TileContext(nc) as tc, ExitStack() as top:
        constp = top.enter_context(tc.tile_pool(name="const", bufs=1))
        wpool = top.enter_context(tc.tile_pool(name="wts", bufs=1))
        pp = top.enter_context(tc.tile_pool(name="persist", bufs=1))

        # --- constants -------------------------------------------------
        # maskKQ[p,q] = MASK_NEG where p > q (keep k<=q in [k,q] layout)
        maskKQ = constp.tile([128, 128], f16, tag="maskKQ")
        nc.gpsimd.memset(maskKQ[:], 0.0)
        nc.gpsimd.affine_select(
            out=maskKQ[:], in_=maskKQ[:], compare_op=Alu.is_ge, fill=MASK_NEG,
            base=0, pattern=[[1, 128]], channel_multiplier=-1)
        # maskQK[q,k] = MASK_NEG where k > q ([q,k] layout)
        maskQK = constp.tile([128, 128], f16, tag="maskQK")
        nc.gpsimd.memset(maskQK[:], 0.0)
        nc.gpsimd.affine_select(
            out=maskQK[:], in_=maskQK[:], compare_op=Alu.is_ge, fill=MASK_NEG,
            base=0, pattern=[[-1, 128]], channel_multiplier=1)
        identM = constp.tile([128, 128], f16, tag="identM")
        masks.make_identity(nc, identM[:])
        ident = constp.tile([128, 128], f32, tag="ident")
        masks.make_identity(nc, ident[:])


        # --- weights (DMA order tuned so tile-0 compute starts early) ---
        wq8 = [wpool.tile([128, n_dc, QROWS], f8, tag=f"wq8_{i}",
                          name=f"wq8_{i}") for i in range(3)]
        wkv8 = [wpool.tile([128, n_dc, 128], f8, tag=f"wkv8_{i}",
                           name=f"wkv8_{i}") for i in range(3)]
        nc.sync.dma_start(
            out=wq8[0][:, :, 0:128],
            in_=wq_d[0].rearrange("(c p) n -> p c n", p=128)[:, :, 0:128])

        def load_weights_rest():
            nc.sync.dma_start(
                out=wq8[0][:, :, 128:QROWS],
                in_=wq_d[0].rearrange("(c p) n -> p c n",
                                      p=128)[:, :, 128:QROWS])
            for i in range(3):
                if i > 0:
                    nc.sync.dma_start(
                        out=wq8[i][:],
                        in_=wq_d[i].rearrange("(c p) n -> p c n", p=128))
                nc.sync.dma_start(
                    out=wkv8[i][:],
                    in_=wkv_d[i].rearrange("(c p) n -> p c n", p=128))

        # o_proj weights: [128, 2(level), d] per contraction chunk; both
        # levels are the same 4*ternary data. Loaded at end of phase 1.
        wo8 = [wpool.tile([128, 2, d], f8, tag=f"wo8_{c}", name=f"wo8_{c}")
               for c in range(n_oc)]

        def load_wo8():
            for c in range(n_oc):
                for lv in range(2):
                    nc.sync.dma_start(
                        out=wo8[c][:, lv, :],
                        in_=wo_d[c * 128:(c + 1) * 128, :])

        # --- persistent activations -----------------------------------
        # qA[h]: rows 0:64 = fp16(q/8), row 64 = m~ bias (max pass)
        # qP[h]: fp8 pack [64, 2, tokens] = (qh/64, qres*64)
        # khb:   rows 0:64 = fp16(k), row 64 = -1
        # kP:    fp8 pack [64, 2, tokens] = (kres*64, kh/64)
        # vhat:  [128, chunk*65]: cols 0:64 of chunk = v, col 64 = 0.25
        qA = [pp.tile([65, tokens], f16, tag=f"qA{h}", name=f"qA{h}")
              for h in range(HEADS_PER_CORE)]
        qP = [pp.tile([64, 2, tokens], f8, tag=f"qP{h}", name=f"qP{h}")
              for h in range(HEADS_PER_CORE)]
        khb = pp.tile([65, tokens], f16, tag="khb")
        kP = pp.tile([64, 2, tokens], f8, tag="kP")
        n_ch = tokens // 128
        vhat = pp.tile([128, n_ch * 65], f32r, tag="vhat")
        nc.scalar.activation(
            vhat[:], ident[:, 0:1].to_broadcast([128, n_ch * 65]),
            Act.Copy, bias=0.25, scale=0.0)
        nc.gpsimd.memset(khb[64:65, :], -1.0)

        with ExitStack() as ph:
            mp = ph.enter_context(tc.tile_pool(name="mp", bufs=3))
            ps1 = ph.enter_context(
                tc.tile_pool(name="ps1", bufs=5, space="PSUM"))
            psst = ph.enter_context(
                tc.tile_pool(name="psst", bufs=2, space="PSUM"))
            psav = ph.enter_context(
                tc.tile_pool(name="psav", bufs=1, space="PSUM"))

            # ---------- S~ max-estimate pass, as schedulable blocks ------
            mstate = {}
            mbp = ph.enter_context(tc.tile_pool(name="mbp", bufs=8))

            def s_block(bb, h, qc):
                boff = bb * s
                if qc == 0:
                    mstate[(bb, h)] = mbp.tile([128, n_qc], f32, tag="mbuf",
                                               name="mbuf")
                mbuf = mstate[(bb, h)]
                qsl = slice(boff + qc * 128, boff + qc * 128 + 128)
                ntk = qc // sub + 1
                mtmp = mp.tile([128, 8], f32, tag="mtmp")
                for kt in range(ntk):
                    w = min(TT, (qc + 1) * 128 - kt * TT)
                    st = psst.tile([128, TT], f32, tag="st")
                    nc.tensor.matmul(
                        st[:, 0:w],
                        lhsT=qA[h][0:64, qsl],
                        rhs=khb[0:64, boff + kt * TT:boff + kt * TT + w],
                        start=True, stop=(kt != ntk - 1))
                    if kt == ntk - 1:  # diagonal block is last 128 cols
                        nc.tensor.matmul(
                            st[:, w - 128:w], lhsT=identM[:], rhs=maskQK[:],
                            start=False, stop=True, skip_group_check=True)
                    nc.vector.tensor_reduce(
                        mtmp[:, kt:kt + 1], st[:, 0:w],
                        axis=mybir.AxisListType.X, op=Alu.max)
                nc.vector.tensor_reduce(
                    mbuf[:, qc:qc + 1], mtmp[:, 0:ntk],
                    axis=mybir.AxisListType.X, op=Alu.max)

            def s_final(bb, h):
                boff = bb * s
                mbuf = mstate.pop((bb, h))
                mps = psst.tile([128, TT], f32, tag="st")
                nc.tensor.transpose(mps[0:n_qc, 0:128], mbuf[:, 0:n_qc],
                                    ident[:, 0:128])
                mrow = mp.tile([n_qc, 128], f32, tag="mrow")
                nc.scalar.copy(mrow[:], mps[0:n_qc, 0:128])
                nc.gpsimd.dma_start(
                    out=qA[h][64:65, boff:boff + s].rearrange(
                        "o (c t) -> o c t", t=128),
                    in_=mrow[:])

            # ================= phase 1: projections ====================
            ph1 = ExitStack()
            xp = ph1.enter_context(tc.tile_pool(name="xp", bufs=2))
            x8p = ph1.enter_context(tc.tile_pool(name="x8p", bufs=2))
            sp1 = ph1.enter_context(tc.tile_pool(name="sp1", bufs=3))
            for tt in range(n_tt):
                tcols = slice(tt * TT, (tt + 1) * TT)
                x8 = [x8p.tile([128, n_dc, TT], f8, tag=f"x8_{i}",
                               name=f"x8_{i}") for i in range(3)]
                for i in range(3):
                    xr = x_d[i].rearrange("(c p) t -> p c t", p=128)
                    for ch in range(2):
                        cs = slice(ch * n_dc // 2, (ch + 1) * n_dc // 2)
                        nc.sync.dma_start(
                            out=x8[i][:, cs], in_=xr[:, cs, tcols])
                if tt == 0:
                    load_weights_rest()

                def proj(w8l, mcol, ps):
                    # each 256-wide half is its own accumulation group
                    for half in range(2):
                        hs = slice(half * 256, half * 256 + 256)
                        for lv in range(3):
                            for cp in range(n_dc // 2):
                                nc.tensor.matmul(
                                    ps[:, hs],
                                    lhsT=w8l[lv][:, 2 * cp:2 * cp + 2,
                                                 mcol:mcol + 128],
                                    rhs=x8[lv][:, 2 * cp:2 * cp + 2, hs],
                                    start=(lv == 0 and cp == 0),
                                    stop=(lv == 2 and cp == n_dc // 2 - 1),
                                    perf_mode=DR,
                                    skip_group_check=(half == 1))
                                first = False

                for m in range(QROWS // 128):
                    ps = ps1.tile([128, TT], f32, tag="ps")
                    proj(wq8, m * 128, ps)
                    for i in range(2):
                        h = 2 * m + i
                        rows = slice(i * 64, i * 64 + 64)
                        # qA = fp16(q/8)
                        nc.scalar.activation(qA[h][0:64, tcols], ps[rows, :],
                                             Act.Copy, scale=0.125)
                        # res = q/8 - qA (fp16: keeps gpsimd inputs 16-bit)
                        res = sp1.tile([64, TT], f16, tag="qres")
                        nc.vector.scalar_tensor_tensor(
                            res[:], in0=ps[rows, :], scalar=0.125,
                            in1=qA[h][0:64, tcols],
                            op0=Alu.mult, op1=Alu.subtract)
                        # fp8 pack (SBUF->SBUF on gpsimd)
                        nc.gpsimd.tensor_scalar_mul(
                            qP[h][:, 0, tcols], qA[h][0:64, tcols],
                            1.0 / PK_SCALE)
                        nc.gpsimd.tensor_scalar_mul(
                            qP[h][:, 1, tcols], res[:], PK_SCALE)

                ps = ps1.tile([128, TT], f32, tag="ps")
                proj(wkv8, 0, ps)
                nc.scalar.copy(khb[0:64, tcols], ps[0:64, :])
                res = sp1.tile([64, TT], f16, tag="qres")
                nc.vector.scalar_tensor_tensor(
                    res[:], in0=ps[0:64, :], scalar=1.0,
                    in1=khb[0:64, tcols], op0=Alu.mult, op1=Alu.subtract)
                nc.gpsimd.tensor_scalar_mul(
                    kP[:, 0, tcols], res[:], PK_SCALE)
                nc.gpsimd.tensor_scalar_mul(
                    kP[:, 1, tcols], khb[0:64, tcols], 1.0 / PK_SCALE)
                vtmp = sp1.tile([64, TT], f32, tag="vtmp")
                nc.scalar.copy(vtmp[:], ps[64:128, :])
                for j in range(sub):
                    ptr = psst.tile([128, TT], f32, tag="st")
                    nc.tensor.transpose(ptr[0:128, 0:64],
                                        vtmp[:, j * 128:(j + 1) * 128],
                                        ident[0:64, 0:64])
                    ch = tt * sub + j
                    nc.scalar.copy(vhat[:, ch * 65:ch * 65 + 64],
                                   ptr[0:128, 0:64])
                # batch-0 S~ blocks for the q-chunks this tile enabled
                bb, ltt = tt // tt_per_b, tt % tt_per_b
                if bb == 0:
                    for h in range(HEADS_PER_CORE):
                        for qc in range(ltt * sub, (ltt + 1) * sub):
                            s_block(bb, h, qc)
                        if ltt == tt_per_b - 1:
                            s_final(bb, h)
                elif ltt < tt_per_b // 2:
                    # cheap half of batch-1 S~ rides the batch-1 proj tiles
                    for h in range(HEADS_PER_CORE):
                        for qc in range(ltt * sub, (ltt + 1) * sub):
                            s_block(bb, h, qc)

            ph1.close()
            # ============ phase 2 + per-batch o_proj ====================
            load_wo8()
            aop = ph.enter_context(tc.tile_pool(name="aop", bufs=1))
            ptp = ph.enter_context(tc.tile_pool(name="ptp", bufs=6))
            outp = ph.enter_context(tc.tile_pool(name="outp", bufs=3))
            # ao[c]: fp8 pack [128, 2(level), tokens]
            ao = [aop.tile([128, 2, tokens], f8, tag=f"ao{i}", name=f"ao{i}")
                  for i in range(n_oc)]

            def av(pav, pt, lo, w, bb, kc, nchunks):
                ch = bb * (s // 128) + kc
                nc.tensor.matmul(
                    pav[:, lo:lo + w], lhsT=vhat[:, ch * 65:ch * 65 + 65],
                    rhs=pt[:, lo:lo + w],
                    start=(kc == 0), stop=(kc == nchunks - 1),
                    skip_group_check=True)

            b1q = []
            for h in range(HEADS_PER_CORE):
                for qc in range(n_qc // 2, n_qc):
                    b1q.append(lambda h=h, qc=qc: s_block(1, h, qc))
                b1q.append(lambda h=h: s_final(1, h))
            # weight S~ pacing by main-slot size (qt+1 chunks of work);
            # slots run qt-major
            wsum = HEADS_PER_CORE * n_qt * (n_qt + 1) // 2
            bcum, acc = [], 0.0
            for qt in range(n_qt):
                for h in range(HEADS_PER_CORE):
                    acc += (qt + 1) * len(b1q) / wsum
                    bcum.append(min(int(round(acc)), len(b1q)))
            bcum[-1] = len(b1q)

            out_r = out_d.rearrange("(mm p) t -> p mm t", p=128)

            def oproj_group(bb, qt, g, eng):
                """4 consecutive m-blocks of one q-tile, one DMA out."""
                boff = bb * s
                osb = outp.tile([128, 4, TT], f16, tag="og", name="osbg")
                for mi in range(4):
                    m = g * 4 + mi
                    po = ps1.tile([128, TT], f32, tag="ps")
                    for half in range(2):
                        hs = slice(half * 256, half * 256 + 256)
                        qsl = slice(boff + qt * TT + half * 256,
                                    boff + qt * TT + half * 256 + 256)
                        for ci in range(n_oc):
                            nc.tensor.matmul(
                                po[:, hs],
                                lhsT=wo8[ci][:, :, m * 128:m * 128 + 128],
                                rhs=ao[ci][:, :, qsl],
                                start=(ci == 0), stop=(ci == n_oc - 1),
                                perf_mode=DR,
                                skip_group_check=(half == 1))
                    dst = osb[:, mi, :]
                    if eng == "v":
                        nc.vector.tensor_copy(dst, po[:])
                    else:
                        nc.scalar.copy(dst, po[:])
                nc.sync.dma_start(
                    out=out_r[:, g * 4:g * 4 + 4,
                              boff + qt * TT:boff + (qt + 1) * TT],
                    in_=osb[:])

            def attn_slot(bb, h, qt):
                boff = bb * s
                qlo = boff + qt * TT
                pav = psav.tile([65, TT], f32, tag="pav")
                nchunks = (qt + 1) * sub
                pipe = []
                for kc in range(nchunks):
                    ksl = slice(boff + kc * 128, boff + kc * 128 + 128)
                    j = kc - qt * sub
                    lo = max(j, 0) * 128  # cols < lo fully masked
                    w = TT - lo
                    s2 = ps1.tile([128, TT], f32, tag="ps")
                    nc.tensor.matmul(
                        s2[:, lo:lo + w], lhsT=khb[:, ksl],
                        rhs=qA[h][:, qlo + lo:qlo + TT],
                        start=True, stop=False)
                    # fp8 DoubleRow lo-pass (<=256-wide halves)
                    nhalf = (w + 255) // 256
                    for hf in range(nhalf):
                        hlo = lo + hf * 256
                        hw = min(256, TT - hlo)
                        nc.tensor.matmul(
                            s2[:, hlo:hlo + hw],
                            lhsT=kP[:, :, ksl],
                            rhs=qP[h][:, :, qlo + hlo:qlo + hlo + hw],
                            start=False,
                            stop=(j < 0 and hf == nhalf - 1),
                            perf_mode=DR, skip_group_check=True)
                    if j >= 0:
                        nc.tensor.matmul(
                            s2[:, lo:lo + 128], lhsT=identM[:],
                            rhs=maskKQ[:], start=False, stop=True,
                            skip_group_check=True)
                    pt = ptp.tile([128, TT], f32r, tag="pt")
                    nc.scalar.activation(pt[:, lo:lo + w],
                                         s2[:, lo:lo + w], Act.Exp)
                    pipe.append((pt, lo, w, kc))
                    if len(pipe) > 3:
                        pv = pipe.pop(0)
                        av(pav, pv[0], pv[1], pv[2], bb, pv[3], nchunks)
                for pv in pipe:
                    av(pav, pv[0], pv[1], pv[2], bb, pv[3], nchunks)
                pipe.clear()

                # pav row 64 = l/4; rec = 4/l; broadcast on gpsimd; then
                # aof = PV * 4/l = 4*attn (fp16). AO0 = fp8(aof/16)
                # = fp8(attn/4), AO1 = fp8(aof/16 - AO0) = attn/4 - AO0.
                # Both wo8 levels are 4*ternary.
                # Pool only sees fp16->fp8 ops; the mixed stt runs on DVE.
                rec = mp.tile([1, TT], f32, tag="rec")
                with nc.allow_low_precision(
                        reason="1/l broadcast feeds fp8 conversions"):
                    nc.vector.reciprocal(rec[:], pav[64:65, :])
                bcs = mp.tile([64, TT], f32, tag="bcs")
                nc.gpsimd.partition_broadcast(bcs[:], rec[:])
                # aof half matches ao's base partition (SB+SB ops
                # require equal base partitions)
                rows = slice((h % 2) * 64, (h % 2) * 64 + 64)
                aof = mp.tile([128, TT], f16, tag="aof")
                nc.vector.tensor_tensor(
                    aof[rows, :], pav[0:64, :], bcs[:], op=Alu.mult)
                c = h // 2
                nc.gpsimd.tensor_scalar_mul(
                    ao[c][rows, 0, qlo:qlo + TT], aof[rows, :], 1.0 / 16)
                nc.vector.scalar_tensor_tensor(
                    ao[c][rows, 1, qlo:qlo + TT],
                    in0=aof[rows, :], scalar=1.0 / 16,
                    in1=ao[c][rows, 0, qlo:qlo + TT],
                    op0=Alu.mult, op1=Alu.subtract)

            # Both batches qt-major: each q-tile's o_proj rides the next
            # q-tile's attention slots. Batch-0 slots also carry the
            # batch-1 S~ rideshare; batch-0's o_proj copies go to ScalarE
            # (DVE is reduce-bound there), batch-1's to DVE (ScalarE is
            # exp-bound there).
            slot = 0
            for qt in range(n_qt):
                for h in range(HEADS_PER_CORE):
                    attn_slot(0, h, qt)
                    lo_i = bcum[slot - 1] if slot else 0
                    for fn in b1q[lo_i:bcum[slot]]:
                        fn()
                    slot += 1
                    if qt > 0:
                        oproj_group(0, qt - 1, h, "s")
            # pending o_proj groups pop one per slot, delayed one slot so
            # the group's ao dependencies never head-block the PE queue
            pend = [(0, n_qt - 1, g, "v") for g in range(HEADS_PER_CORE)]
            for qt in range(n_qt):
                for h in range(HEADS_PER_CORE):
                    attn_slot(1, h, qt)
                    if qt > 0 or h > 0:
                        oproj_group(*pend.pop(0))
                pend += [(1, qt, g, "v") for g in range(HEADS_PER_CORE)]
            for args in pend:
                oproj_group(*args)

    nc.compile()
    return nc


def _ternarize(w):
    w = np.asarray(w, np.float32)
    scale = max(np.abs(w).mean(), 1e-6)
    return ((w > 0.05 * scale).astype(np.float32)
            - (w < -0.05 * scale).astype(np.float32))


def kernel(x, wq, wk, wv, wo):
    from concourse.bass_utils import run_bass_kernel_spmd

    if "nc" not in _CACHE:
        _CACHE["nc"] = _build_program()
    nc = _CACHE["nc"]

    tq = _ternarize(wq)
    tk = _ternarize(wk)
    tv = _ternarize(wv)
    to = _ternarize(wo)

    xT = np.ascontiguousarray(np.asarray(x, np.float32).reshape(B * S, D).T)
    # 3-level e4m3 split of x; weight copies at 1, 1/32, 1/512 (all exact)
    x0 = xT.astype(E4)
    r1 = xT - x0.astype(np.float32)
    x1 = (r1 * 32.0).astype(E4)
    r2 = r1 - x1.astype(np.float32) / 32.0
    x2 = (r2 * 512.0).astype(E4)
    xs = [x0, x1, x2]
    scales = [1.0, 1.0 / 32, 1.0 / 512]

    in_maps = []
    for c in range(NCORES):
        qsl = slice(c * QROWS, (c + 1) * QROWS)
        ksl = slice(c * HD, (c + 1) * HD)
        wkv = np.concatenate([tk[ksl], tv[ksl]], axis=0)  # [128, D]
        wqT = np.ascontiguousarray(tq[qsl].T)
        wkvT = np.ascontiguousarray(wkv.T)
        m = {"wo8": np.ascontiguousarray(to[:, qsl].T * 4.0).astype(E4)}
        for i in range(3):
            m[f"x{i}"] = xs[i]
            m[f"wq{i}"] = (wqT * scales[i]).astype(E4)
            m[f"wkv{i}"] = (wkvT * scales[i]).astype(E4)
        in_maps.append(m)

    res = run_bass_kernel_spmd(nc, in_maps, list(range(NCORES)))
    total = res.results[0]["out"].astype(np.float32)
    for c in range(1, NCORES):
        total = total + res.results[c]["out"].astype(np.float32)
    return np.ascontiguousarray(total.T).reshape(B, S, D).astype(np.float32)
